# revision 1
# baseline (speedup 1.0000x reference)
"""GQA attention kernel v2 for Trainium2, 8 NeuronCores.

Problem: B=4, T=2048, C=2048, H=16 q-heads, KVH=4 kv-heads, D=128, RoPE,
causal mask, out-projection with bias.

Sharding: (batch b, q-head-group g) -> core 2*b+g. Each core handles one
batch and 8 q heads (= 2 kv heads). The out-projection partial sums of the
two head-groups of a batch are summed on the host (+ bias).

v2 changes vs v1:
  - all matmul operands bf16 (same PE rate as f32r, half DMA/SBUF)
  - sin/cos tables + canonical causal mask tiles precomputed on host
  - weights fully SBUF-resident (loaded once)
  - diagonal-band score blocks trimmed to the valid q range
  - out-projection matmul chains interleaved into the softmax pipeline to
    fill PE bubbles while ACT computes exp
  - softmax epilogue: reciprocal straight from PSUM, gpsimd broadcast

Per-core dataflow (feature-major "T" layouts):
  xT  [C, T]          activations, C on partitions (16 chunks of 128)
  QT  [128, 8, T]     q projections, partition = d within head (bf16, RoPE'd)
  KT  [128, 2, T]     keys (bf16, RoPE'd)
  V   [128, NKT, 256] values, partition = token within 128-tile (bf16)
  S^T [Tk, Tq]        scores computed transposed; softmax along partitions:
                      exp on ACT, column sums via ones-matmul on PE,
                      normalization deferred to O^T (divide by l).
  OT  [128, 8, 512]   attention outputs per group, partition = d
  out [T, C]          partial out-projection (fp32)
"""

import math

import numpy as np

B, T, C = 4, 2048, 2048
H, KVH, D = 16, 4, 128
HPC = 8      # q heads per core
KVPC = 2     # kv heads per core
P = 128

_compiled = {}


def _build_causal(seq=T):
    import concourse.bacc as bacc
    import concourse.mybir as mybir
    import concourse.tile as tile

    f32 = mybir.dt.float32
    bf16 = mybir.dt.bfloat16
    AF = mybir.ActivationFunctionType
    ALU = mybir.AluOpType

    NKT = seq // P          # k tiles of 128
    NG = seq // 512         # q groups of 512
    KC = C // P             # 16 contraction chunks
    RPH = HPC // KVPC       # q heads per kv head

    nc = bacc.Bacc(None, target_bir_lowering=False)

    xT = nc.dram_tensor("xT", [C, seq], bf16, kind="ExternalInput")
    wqt = nc.dram_tensor("wqt", [HPC, P, KC * P], bf16, kind="ExternalInput")
    wkT = nc.dram_tensor("wkT", [C, KVPC * D], bf16, kind="ExternalInput")
    wvT = nc.dram_tensor("wvT", [C, KVPC * D], bf16, kind="ExternalInput")
    woT = nc.dram_tensor("woT", [HPC * D, C], bf16, kind="ExternalInput")
    cmask = nc.dram_tensor("cmask", [P, 4, 512], bf16, kind="ExternalInput")
    sinT = nc.dram_tensor("sinT", [D, seq], bf16, kind="ExternalInput")
    cosT = nc.dram_tensor("cosT", [D, seq], bf16, kind="ExternalInput")
    out = nc.dram_tensor("out", [seq, C], f32, kind="ExternalOutput")

    xT_r = xT.rearrange("(kc p) t -> p kc t", p=P)
    wkT_r = wkT.rearrange("(kc p) m -> p kc m", p=P)
    wvT_r = wvT.rearrange("(kc p) m -> p kc m", p=P)
    woT_r = woT.rearrange("(h p) c -> p h c", p=P)

    with (
        tile.TileContext(nc) as tc,
        tc.tile_pool(name="persist", bufs=1) as persist,
        tc.tile_pool(name="xh", bufs=2) as xhp,
    ):
        QT = persist.tile([P, HPC, seq], bf16)
        KT = persist.tile([P, KVPC, seq], bf16)
        V = persist.tile([P, NKT, KVPC * D], bf16)
        sin_t = persist.tile([P, seq], bf16)
        cos_t = persist.tile([P, seq], bf16)
        cm = persist.tile([P, 4, 512], bf16)
        ones = persist.tile([P, 1], bf16)
        wks = persist.tile([P, KC, KVPC * D], bf16)
        wvs = persist.tile([P, KC, KVPC * D], bf16)
        wos = persist.tile([P, HPC, C], bf16)

        nc.vector.memset(ones[:], 1.0)

        # ======== phase 1: projections + RoPE ========
        NQ = seq // 512
        DEFER_V = set(range(max(1, NQ - 2), NQ))
        xq_tiles = {}
        with (
            tc.tile_pool(name="wm", bufs=5) as wmp,
            tc.tile_pool(name="qrp", bufs=4) as qrp,
            tc.tile_pool(name="qsp", bufs=3) as qsp,
            tc.tile_pool(name="rtp", bufs=3) as rtp,
            tc.tile_pool(name="psq", bufs=4, space="PSUM") as psqp,
            tc.tile_pool(name="psv", bufs=2, space="PSUM") as psvp,
        ):
            for hf in range(NQ):
                gch = slice(hf * 512, (hf + 1) * 512)
                xq = xhp.tile([P, KC, 512], bf16, tag="xq")
                xq_tiles[hf] = xq

                def dma_xq():
                    # split so the first chunks land early and matmuls can
                    # start before the whole tile arrives
                    for kc0 in range(0, KC, 4):
                        nc.sync.dma_start(
                            xq[:, kc0:kc0 + 4, :],
                            xT_r[:, kc0:kc0 + 4, gch])

                def dma_wq(m):
                    wa = wmp.tile([P, KC, P], bf16, tag="wm")
                    nc.sync.dma_start(
                        wa[:], wqt[m].rearrange("p (kc c) -> p kc c", c=P))
                    return wa

                if hf > 0:
                    # out-projection weights: needed only by the attention
                    # phase; stream them in chunks on the ACT ring, gated
                    # behind each x-chunk's ACT work so they stay clear of
                    # the phase-1 SP DMA burst
                    for hh in range((hf - 1) * HPC // NQ, hf * HPC // NQ):
                        nc.scalar.dma_start(
                            wos[:, hh:hh + 1, :], woT_r[:, hh:hh + 1, :])
                if hf == 0:
                    # critical-path order: first weights, first x chunks,
                    # then tables and the resident K/V weights
                    wq_pre = {0: dma_wq(0)}
                    for kc0 in range(0, KC, 4):
                        nc.sync.dma_start(
                            xq[:, kc0:kc0 + 4, :],
                            xT_r[:, kc0:kc0 + 4, gch])
                        if kc0 == 0:
                            wq_pre[1] = dma_wq(1)
                        if kc0 == 8:
                            wq_pre[2] = dma_wq(2)
                    wq_pre[3] = dma_wq(3)
                    wq_pre[4] = dma_wq(4)
                    nc.sync.dma_start(sin_t[:], sinT[:])
                    nc.sync.dma_start(cos_t[:], cosT[:])
                    # run the first two head chains column-major over kc
                    # groups so PE tracks the arriving x chunks
                    ps01 = []
                    for _i in range(2):
                        psq_cm = psqp.tile([P, 512], f32, tag="pq")
                        ps01.append(psq_cm)
                    for kc0 in range(0, KC, 4):
                        for m in range(2):
                            wa = wq_pre[m]
                            for kc in range(kc0, kc0 + 4):
                                nc.tensor.matmul(
                                    ps01[m][:], wa[:, kc, :], xq[:, kc, :],
                                    start=(kc == 0), stop=(kc == KC - 1))
                    wq_head = {0: ps01[0], 1: ps01[1]}
                else:
                    dma_xq()
                    wq_pre = {}
                    wq_head = {}

                # Q (streamed weights) then K (resident) projections + RoPE
                for nm, ws, dstT in ((HPC, None, QT), (KVPC, wks, KT)):
                    for m in range(nm):
                        if ws is None and hf == 0:
                            if m + 2 < HPC and (m + 2) not in wq_pre:
                                wq_pre[m + 2] = dma_wq(m + 2)
                            if m == HPC - 2:
                                nc.sync.dma_start(wks[:], wkT_r[:])
                                nc.sync.dma_start(wvs[:], wvT_r[:])
                                nc.sync.dma_start(cm[:], cmask[:])
                        if ws is None and m in wq_head:
                            psq = wq_head[m]
                        else:
                            if ws is None:
                                if m in wq_pre:
                                    wa = wq_pre[m]
                                else:
                                    wa = dma_wq(m)
                            psq = psqp.tile([P, 512], f32, tag="pq")
                            for kc in range(KC):
                                if ws is None:
                                    wt = wa[:, kc, :]
                                else:
                                    wt = ws[:, kc, m * P:(m + 1) * P]
                                nc.tensor.matmul(
                                    psq[:], wt, xq[:, kc, :],
                                    start=(kc == 0), stop=(kc == KC - 1))
                        qr = qrp.tile([P, 512], bf16, tag="qr")
                        nc.scalar.activation(qr[:], psq[:], AF.Copy)
                        qs = qsp.tile([P, 512], bf16, tag="qs")
                        nc.scalar.dma_start(qs[0:64, :], qr[64:128, :])
                        nc.scalar.dma_start(qs[64:128, :], qr[0:64, :])
                        rt = rtp.tile([P, 512], bf16, tag="rt")
                        nc.vector.tensor_tensor(
                            rt[:], qs[:], sin_t[:, gch], ALU.mult)
                        nc.vector.tensor_tensor(
                            dstT[:, m, gch], qr[:], cos_t[:, gch], ALU.mult)
                        nc.vector.tensor_tensor(
                            dstT[:, m, gch], dstT[:, m, gch], rt[:], ALU.add)

                # V projection; the last chunks are deferred into the start
                # of the attention phase to fill its PE bubbles
                if hf not in DEFER_V:
                    for tt in range(4):
                        gtt = hf * 4 + tt
                        psv = psvp.tile([P, KVPC * D], f32, tag="pv")
                        for kc in range(KC):
                            nc.tensor.matmul(
                                psv[:], xq[:, kc, tt * P:(tt + 1) * P],
                                wvs[:, kc, :],
                                start=(kc == 0), stop=(kc == KC - 1))
                        nc.scalar.activation(V[:, gtt, :], psv[:], AF.Copy)

        start_h = (NQ - 1) * HPC // NQ if NQ > 1 else 0
        for hh in range(start_h, HPC):
            nc.scalar.dma_start(
                wos[:, hh:hh + 1, :], woT_r[:, hh:hh + 1, :])

        # ======== phases 2+3: attention + interleaved out-projection ====
        # Chunk order: groups 0 and 1 interleaved (g0 alone is too small to
        # keep PE busy), then groups 2, 3. Out-projection chains for group g
        # are injected into later chunks to fill exp-latency PE bubbles.
        # The softmax denominator is summed in quads on DVE (bf16 2x mode)
        # so the PE ones-matmul runs once per quad instead of per block.
        with (
            tc.tile_pool(name="otp", bufs=3) as otp,
            tc.tile_pool(name="pch", bufs=7) as pch,
            tc.tile_pool(name="lacc", bufs=2) as laccp,
            tc.tile_pool(name="lbp", bufs=2) as lbp,
            tc.tile_pool(name="sm2", bufs=4) as sm2,
            tc.tile_pool(name="obp", bufs=3) as obp,
            tc.tile_pool(name="ps_s", bufs=3, space="PSUM") as ps_s,
            tc.tile_pool(name="ps_o", bufs=2, space="PSUM") as ps_o,
            tc.tile_pool(name="ps_l", bufs=1, space="PSUM") as ps_l,
            tc.tile_pool(name="ps3", bufs=2, space="PSUM") as ps3p,
        ):
            OTg = {}        # group -> OT tile [P, HPC, 512]
            pending = []    # deferred PE work (closures)

            def chain_mms(g, tt, co, ps3, hs):
                csl = slice(co * 512, (co + 1) * 512)
                for h in hs:
                    nc.tensor.matmul(
                        ps3[:], OTg[g][:, h, tt * P:(tt + 1) * P],
                        wos[:, h, csl],
                        start=(h == 0), stop=(h == HPC - 1))

            def chain_evac(g, tt, co, ps3):
                csl = slice(co * 512, (co + 1) * 512)
                ob = obp.tile([P, 512], f32, tag="ob")
                nc.vector.tensor_copy(ob[:], ps3[:])
                nc.sync.dma_start(
                    out[(4 * g + tt) * P:(4 * g + tt + 1) * P, csl], ob[:])

            def emit_chain(g, tt, co):
                """One out-projection chain: 8 matmuls + evacuate + DMA."""
                ps3 = ps3p.tile([P, 512], f32, tag="po")
                chain_mms(g, tt, co, ps3, range(HPC))
                chain_evac(g, tt, co, ps3)

            def emit_vchain(hf, tt):
                """Deferred V projection for one 128-token tile."""
                xq = xq_tiles[hf]
                gtt = hf * 4 + tt
                psv = ps3p.tile([P, KVPC * D], f32, tag="po")
                for kc in range(KC):
                    nc.tensor.matmul(
                        psv[:], xq[:, kc, tt * P:(tt + 1) * P],
                        wvs[:, kc, :],
                        start=(kc == 0), stop=(kc == KC - 1))
                nc.scalar.activation(V[:, gtt, :], psv[:], AF.Copy)

            def drain(k):
                for _ in range(min(k, len(pending))):
                    kind, args = pending.pop(0)
                    if kind == "v":
                        emit_vchain(*args)
                    else:
                        emit_chain(*args)

            for dhf in sorted(DEFER_V):
                for tt in range(4):
                    pending.append(("v", (dhf, tt)))

            order = [(g, h) for g in range(NG) for h in range(HPC)]

            # chunk state: (g, h, pso, psl, blocks emitted so far)
            class Chunk:
                def __init__(self, g, h):
                    self.g, self.h = g, h
                    self.kv = h // RPH
                    self.ntk = 4 * (g + 1)
                    self.pso = ps_o.tile([P, 512], f32, tag="o")
                    self.psl = ps_l.tile([1, 512], f32, tag="l")
                    self.pcs = {}          # block -> pc tile

                def q0_of(self, j):
                    return 128 * max(0, j - 4 * self.g)

                def emit_s(self, j):
                    q0 = self.q0_of(j)
                    pss = ps_s.tile([P, 512], f32, tag="s")
                    nc.tensor.matmul(
                        pss[:, q0:], KT[:, self.kv, j * P:(j + 1) * P],
                        QT[:, self.h, self.g * 512 + q0:(self.g + 1) * 512],
                        start=True, stop=True)
                    u = j - 4 * self.g
                    if u >= 0:
                        # mask is nontrivial only on the diagonal 128 cols
                        nc.vector.tensor_tensor(
                            pss[:, q0:q0 + P], pss[:, q0:q0 + P],
                            cm[:, u, q0:q0 + P], ALU.add)
                    return pss

                def emit_lo(self, i, pss):
                    """exp, O-matmul, and running l-accumulation for block
                    i. All full blocks are summed into one DVE accumulator
                    (bf16) so the PE ones-matmul runs once per chunk; the
                    diagonal blocks (trimmed widths) are summed in pairs."""
                    g, ntk = self.g, self.ntk
                    q0 = self.q0_of(i)
                    pc = pch.tile([P, 512], bf16, tag="p")
                    nc.scalar.activation(pc[:, q0:], pss[:, q0:], AF.Exp)
                    self.pcs[i] = pc
                    nc.tensor.matmul(
                        self.pso[:, q0:], V[:, i, self.kv * D:(self.kv + 1) * D],
                        pc[:, q0:], start=(i == 0), stop=(i == ntk - 1))
                    nfull = 4 * g
                    if g > 0:
                        # one running bf16 accumulator for the whole chunk
                        # (diagonal blocks fold into their trimmed region),
                        # then a single ones-matmul
                        if i == 1:
                            lacc = laccp.tile([P, 512], bf16, tag="la")
                            self.lacc = lacc
                            nc.vector.tensor_tensor(
                                lacc[:], self.pcs.pop(0)[:], pc[:], ALU.add)
                            self.pcs.pop(1, None)
                        elif i > 1:
                            nc.vector.tensor_tensor(
                                self.lacc[:, q0:], self.lacc[:, q0:],
                                pc[:, q0:], ALU.add)
                            self.pcs.pop(i, None)
                        if i == ntk - 1:
                            nc.tensor.matmul(
                                self.psl[:], ones[:], self.lacc[:],
                                start=True, stop=True)
                    else:
                        u = i
                        if u == 1:
                            lead = self.pcs[0]
                            nc.vector.tensor_tensor(
                                lead[:, P:], lead[:, P:], pc[:, P:], ALU.add)
                            nc.tensor.matmul(
                                self.psl[:], ones[:], lead[:],
                                start=True, stop=False)
                        elif u == 3:
                            lead = self.pcs[2]
                            nc.vector.tensor_tensor(
                                lead[:, 3 * P:], lead[:, 3 * P:], pc[:, 3 * P:],
                                ALU.add)
                            nc.tensor.matmul(
                                self.psl[:, 2 * P:], ones[:], lead[:, 2 * P:],
                                start=False, stop=True)

                def epilogue(self):
                    # psl -> bf16 on ACT so the reciprocal runs in the DVE
                    # 2x mode (l only normalizes, bf16 precision is plenty)
                    ls = sm2.tile([1, 512], bf16, tag="ls")
                    nc.scalar.activation(ls[:], self.psl[:], AF.Copy)
                    lrc = sm2.tile([1, 512], bf16, tag="lrc")
                    with nc.allow_low_precision(
                            reason="softmax denom reciprocal in bf16"):
                        nc.vector.reciprocal(lrc[:], ls[:])
                    lb = lbp.tile([P, 512], bf16, tag="lb")
                    nc.gpsimd.partition_broadcast(lb[:], lrc[:])
                    nc.vector.tensor_tensor(
                        OTg[self.g][:, self.h, :], self.pso[:], lb[:],
                        ALU.mult)

            # flat block stream with cross-chunk S-prefetch (depth PF)
            PF = 2
            stream = []
            for g, h in order:
                if g not in OTg:
                    ot_g = otp.tile([P, HPC, 512], bf16, tag="ot")
                    OTg[g] = ot_g
                stream.append((g, h))

            chunks = {}
            flat = []        # (chunk, local block index)
            for g, h in stream:
                ck = None
                for j in range(4 * (g + 1)):
                    flat.append(((g, h), j))

            done_heads = {g: 0 for g in range(NG)}
            live = {}
            spos = 0

            def ensure_chunk(key):
                if key not in live:
                    live[key] = Chunk(*key)
                return live[key]

            def emit_s_at(idx):
                key, j = flat[idx]
                ck = ensure_chunk(key)
                return ck, ck.emit_s(j)

            squeue = []
            for _ in range(min(PF, len(flat))):
                squeue.append(emit_s_at(spos))
                spos += 1
            drain(1)
            for idx in range(len(flat)):
                key, j = flat[idx]
                ck, pss = squeue.pop(0)
                assert (ck.g, ck.h) == key and True
                ck.emit_lo(j, pss)
                if spos < len(flat):
                    nkey = flat[spos][0]
                    squeue.append(emit_s_at(spos))
                    spos += 1
                if j == ck.ntk - 1:
                    ck.epilogue()
                    del live[key]
                    g = key[0]
                    done_heads[g] += 1
                    if done_heads[g] == HPC:
                        for tt in range(4):
                            for co in range(C // 512):
                                pending.append(("c", (g, tt, co)))
                    drain(1)
                elif j == ck.ntk - 3:
                    # keep chains in reserve so the last chunks' exp tails
                    # still have PE fill work
                    if idx >= len(flat) - 2 * ck.ntk:
                        drain(2)
                    elif len(pending) > 5:
                        drain(1)
            if len(pending) >= 2 and pending[0][0] == "c" \
                    and pending[1][0] == "c":
                a = pending.pop(0)[1]
                b = pending.pop(0)[1]
                psa = ps3p.tile([P, 512], f32, tag="po")
                psb = ps3p.tile([P, 512], f32, tag="po")
                chain_mms(*a, psa, range(HPC - 1))
                chain_mms(*b, psb, range(HPC - 1))
                chain_mms(*a, psa, [HPC - 1])
                chain_evac(*a, psa)
                chain_mms(*b, psb, [HPC - 1])
                chain_evac(*b, psb)
            drain(len(pending))

    nc.finalize()
    return nc


def _build_generic(seq=T, causal=False):
    import concourse.bacc as bacc
    import concourse.mybir as mybir
    import concourse.tile as tile

    f32 = mybir.dt.float32
    f32r = mybir.dt.float32r
    i32 = mybir.dt.int32
    AF = mybir.ActivationFunctionType
    ALU = mybir.AluOpType

    NKT = seq // P          # Tk tiles of 128
    NG = seq // 512         # Tq groups of 512
    KC = C // P             # 16 contraction chunks
    RPH = HPC // KVPC       # q heads per kv head

    nc = bacc.Bacc(None, target_bir_lowering=False)

    xT = nc.dram_tensor("xT", [C, seq], f32r, kind="ExternalInput")
    wqT = nc.dram_tensor("wqT", [C, HPC * D], f32r, kind="ExternalInput")
    wkT = nc.dram_tensor("wkT", [C, KVPC * D], f32r, kind="ExternalInput")
    wvT = nc.dram_tensor("wvT", [C, KVPC * D], f32r, kind="ExternalInput")
    woT = nc.dram_tensor("woT", [HPC * D, C], f32r, kind="ExternalInput")
    maskT = nc.dram_tensor("maskT", [seq, seq], f32, kind="ExternalInput")
    ifT = nc.dram_tensor("ifT", [D, seq], f32, kind="ExternalInput")
    rmat = nc.dram_tensor("rmat", [D, D], f32r, kind="ExternalInput")
    out = nc.dram_tensor("out", [seq, C], f32, kind="ExternalOutput")

    xT_r = xT.rearrange("(kc p) t -> p kc t", p=P)
    wqT_r = wqT.rearrange("(kc p) m -> p kc m", p=P)
    wkT_r = wkT.rearrange("(kc p) m -> p kc m", p=P)
    wvT_r = wvT.rearrange("(kc p) m -> p kc m", p=P)
    woT_r = woT.rearrange("(h p) c -> p h c", p=P)

    with (
        tile.TileContext(nc) as tc,
        tc.tile_pool(name="persist", bufs=1) as persist,
        tc.tile_pool(name="small", bufs=4) as small,
    ):
        QT = persist.tile([P, HPC, seq], f32r)
        KT = persist.tile([P, KVPC, seq], f32r)
        V = persist.tile([P, NKT, KVPC * D], f32r)
        rm = persist.tile([P, D], f32r)
        nc.sync.dma_start(rm[:], rmat[:])
        ones32 = small.tile([P, 1], f32)
        nc.vector.memset(ones32[:], 1.0)
        ones = persist.tile([P, 1], f32r)
        nc.vector.tensor_copy(ones[:], ones32[:])

        # ======== phase 1: trig tables, projections, RoPE ========
        NH = seq // 1024 if seq >= 1024 else 1
        HL = seq // NH  # half length
        with tc.tile_pool(name="ph1", bufs=1) as ph1:
            sinT = ph1.tile([P, seq], f32)
            cosT = ph1.tile([P, seq], f32)
            wv_sb = ph1.tile([P, KC, KVPC * D], f32r)

            # sin/cos tables via range-reduced LUT sin:
            # f = (theta/2pi + shift) mod 1;  sin(2*pi*f)
            with tc.tile_pool(name="trig", bufs=1) as trig:
                tf_ = trig.tile([P, seq], f32, tag="tf")
                nc.sync.dma_start(tf_[:], ifT[:])
                inv2pi = float(1.0 / (2.0 * math.pi))
                for dst, shift in ((sinT, 0.0), (cosT, 0.25)):
                    ty = trig.tile([P, seq], f32, tag="ty")
                    nc.vector.tensor_scalar_mul(ty[:], tf_[:], inv2pi)
                    if shift:
                        nc.vector.tensor_scalar_add(ty[:], ty[:], shift)
                    ti_ = trig.tile([P, seq], i32, tag="ti")
                    nc.vector.tensor_copy(ti_[:], ty[:])
                    tfr = trig.tile([P, seq], f32, tag="tfr")
                    nc.vector.tensor_copy(tfr[:], ti_[:])
                    nc.vector.tensor_tensor(ty[:], ty[:], tfr[:], ALU.subtract)
                    nc.scalar.activation(dst[:], ty[:], AF.Sin,
                                         scale=float(2.0 * math.pi))

            with (
                tc.tile_pool(name="xh", bufs=2) as xhp,
                tc.tile_pool(name="wm", bufs=3) as wmp,
                tc.tile_pool(name="praw", bufs=1) as praw,
                tc.tile_pool(name="ps1", bufs=2, space="PSUM") as ps1,
            ):
                NQ = seq // 512
                for hf in range(NQ):
                    gch = slice(hf * 512, (hf + 1) * 512)
                    xq = xhp.tile([P, KC, 512], f32r, tag="xq")
                    nc.sync.dma_start(xq[:], xT_r[:, :, gch])
                    if hf == 0:
                        nc.sync.dma_start(wv_sb[:], wvT_r[:])

                    # Q then K projections + RoPE
                    for nm, wr, dstT in ((HPC, wqT_r, QT), (KVPC, wkT_r, KT)):
                        for m in range(nm):
                            wa = wmp.tile([P, KC // 2, P], f32r, tag="wm")
                            nc.sync.dma_start(
                                wa[:], wr[:, :KC // 2, m * P:(m + 1) * P])
                            wb = wmp.tile([P, KC // 2, P], f32r, tag="wm")
                            nc.sync.dma_start(
                                wb[:], wr[:, KC // 2:, m * P:(m + 1) * P])
                            psq = ps1.tile([P, 512], f32, tag="pq")
                            for kc in range(KC):
                                wt = wa if kc < KC // 2 else wb
                                nc.tensor.matmul(
                                    psq[:], wt[:, kc % (KC // 2), :],
                                    xq[:, kc, :],
                                    start=(kc == 0), stop=(kc == KC - 1))
                            qr = praw.tile([P, 512], f32r, tag="qr")
                            nc.scalar.activation(qr[:], psq[:], AF.Copy)
                            psr = ps1.tile([P, 512], f32, tag="pr")
                            nc.tensor.matmul(psr[:], rm[:], qr[:],
                                             start=True, stop=True)
                            # dst = qr*cos + rot*sin (rot*sin in-place in PSUM)
                            nc.vector.tensor_tensor(
                                psr[:], psr[:], sinT[:, gch], ALU.mult)
                            nc.vector.tensor_tensor(
                                dstT[:, m, gch], qr[:].bitcast(f32),
                                cosT[:, gch], ALU.mult)
                            nc.vector.tensor_tensor(
                                dstT[:, m, gch],
                                dstT[:, m, gch].bitcast(f32), psr[:],
                                ALU.add)

                    # V projection
                    for tt in range(4):
                        gtt = hf * 4 + tt
                        psv = ps1.tile([P, KVPC * D], f32, tag="pv")
                        for kc in range(KC):
                            nc.tensor.matmul(
                                psv[:], xq[:, kc, tt * P:(tt + 1) * P],
                                wv_sb[:, kc, :],
                                start=(kc == 0), stop=(kc == KC - 1))
                        nc.scalar.activation(V[:, gtt, :], psv[:], AF.Copy)

        # ======== phases 2+3 ========
        with tc.tile_pool(name="otp", bufs=1) as otp:
            OT = otp.tile([P, HPC, seq], f32r)

            with (
                tc.tile_pool(name="mb", bufs=2) as mbp,
                tc.tile_pool(name="pch", bufs=3) as pch,
                tc.tile_pool(name="lbp", bufs=2) as lbp,
                tc.tile_pool(name="sm2", bufs=4) as sm2,
                tc.tile_pool(name="ps_s", bufs=4, space="PSUM") as ps_s,
                tc.tile_pool(name="ps_o", bufs=2, space="PSUM") as ps_o,
                tc.tile_pool(name="ps_l", bufs=2, space="PSUM") as ps_l,
            ):
                for g in range(NG):
                    qsl = slice(g * 512, (g + 1) * 512)
                    nb = 4 if causal else NKT
                    i0 = 4 * g if causal else 0
                    ntk = 4 * (g + 1) if causal else NKT
                    mb = mbp.tile([P, nb, 512], f32, tag="mb")
                    nc.sync.dma_start(
                        mb[:],
                        maskT[i0 * P:(i0 + nb) * P, qsl].rearrange(
                            "(i p) t -> p i t", p=P))
                    for h in range(HPC):
                        kv = h // RPH
                        pso = ps_o.tile([P, 512], f32, tag="o")
                        psl = ps_l.tile([1, 512], f32, tag="l")

                        # software pipeline: keep 2 S-matmuls in flight ahead
                        # of the exp-dependent l/O matmuls so the PE never
                        # stalls on the DVE-mask -> ACT-exp chain.
                        def emit_s(j):
                            pss = ps_s.tile([P, 512], f32, tag="s")
                            nc.tensor.matmul(
                                pss[:], KT[:, kv, j * P:(j + 1) * P],
                                QT[:, h, qsl], start=True, stop=True)
                            if j >= i0:
                                nc.vector.tensor_tensor(
                                    pss[:], pss[:], mb[:, j - i0, :], ALU.add)
                            return pss

                        sq = [emit_s(j) for j in range(min(2, ntk))]
                        for i in range(ntk):
                            pss = sq.pop(0)
                            pc = pch.tile([P, 512], f32r, tag="p")
                            nc.scalar.activation(pc[:], pss[:], AF.Exp)
                            if i + 2 < ntk:
                                sq.append(emit_s(i + 2))
                            nc.tensor.matmul(
                                psl[:], ones[:], pc[:],
                                start=(i == 0), stop=(i == ntk - 1))
                            nc.tensor.matmul(
                                pso[:], V[:, i, kv * D:(kv + 1) * D], pc[:],
                                start=(i == 0), stop=(i == ntk - 1))
                        lsb = sm2.tile([1, 512], f32, tag="lsb")
                        nc.vector.tensor_copy(lsb[:], psl[:])
                        lrc = sm2.tile([1, 512], f32, tag="lrc")
                        nc.vector.reciprocal(lrc[:], lsb[:])
                        lb = lbp.tile([P, 512], f32, tag="lb")
                        nc.gpsimd.partition_broadcast(lb[:], lrc[:])
                        nc.vector.tensor_tensor(
                            OT[:, h, qsl], pso[:], lb[:], ALU.mult)

            # out-projection, co-chunk outer so weight slices stream once
            with (
                tc.tile_pool(name="wo", bufs=12) as wop,
                tc.tile_pool(name="ob", bufs=3) as obp,
                tc.tile_pool(name="ps3", bufs=4, space="PSUM") as ps3,
            ):
                for co in range(C // 512):
                    csl = slice(co * 512, (co + 1) * 512)
                    woh = []
                    for h in range(HPC):
                        w = wop.tile([P, 512], f32r, tag="wo")
                        nc.sync.dma_start(w[:], woT_r[:, h, csl])
                        woh.append(w)
                    for tt in range(NKT):
                        pso3 = ps3.tile([P, 512], f32, tag="po")
                        for h in range(HPC):
                            nc.tensor.matmul(
                                pso3[:], OT[:, h, tt * P:(tt + 1) * P],
                                woh[h][:],
                                start=(h == 0), stop=(h == HPC - 1))
                        ob = obp.tile([P, 512], f32, tag="ob")
                        nc.scalar.activation(ob[:], pso3[:], AF.Copy)
                        nc.sync.dma_start(
                            out[tt * P:(tt + 1) * P, csl], ob[:])

    nc.finalize()
    return nc




def _prep_in_maps_generic(x, inv_freqs, mask, Wq, Wk, Wv, Wo, seq):
    scale = 1.0 / math.sqrt(D)
    maskT = np.ascontiguousarray(mask.reshape(seq, seq).T)
    ifT = np.ascontiguousarray(inv_freqs.reshape(seq, D).T)
    rmat = _rope_rmat()

    shard = []
    for g in range(2):
        wqT = np.ascontiguousarray((Wq[g * 1024:(g + 1) * 1024, :] * scale).T)
        wkT = np.ascontiguousarray(Wk[g * 256:(g + 1) * 256, :].T)
        wvT = np.ascontiguousarray(Wv[g * 256:(g + 1) * 256, :].T)
        woT = np.ascontiguousarray(Wo[:, g * 1024:(g + 1) * 1024].T)
        shard.append((wqT, wkT, wvT, woT))

    in_maps = []
    for b in range(B):
        xTb = np.ascontiguousarray(x[b].T)
        for g in range(2):
            wqT, wkT, wvT, woT = shard[g]
            in_maps.append({
                "xT": xTb, "wqT": wqT, "wkT": wkT, "wvT": wvT, "woT": woT,
                "maskT": maskT, "ifT": ifT, "rmat": rmat,
            })
    return in_maps




def _get_compiled(seq, causal):
    key = (seq, causal)
    if key not in _compiled:
        if causal:
            _compiled[key] = _build_causal(seq)
        else:
            _compiled[key] = _build_generic(seq)
    return _compiled[key]


def _rope_rmat():
    # lhsT for rot = Pmat @ q, Pmat[2i, 2i+1] = -1, Pmat[2i+1, 2i] = 1:
    # lhsT[d', d] = Pmat[d, d']
    m = np.zeros((D, D), dtype=np.float32)
    for i in range(D // 2):
        m[2 * i + 1, 2 * i] = -1.0
        m[2 * i, 2 * i + 1] = 1.0
    return m


def _canonical_mask():
    # cmask[p, u, t] = 0 if 128u + p <= t else -1e9, for t in [0, 512)
    p = np.arange(P)[:, None, None]
    u = np.arange(4)[None, :, None]
    t = np.arange(512)[None, None, :]
    return np.where(128 * u + p <= t, 0.0, -1e9).astype(np.float32)


def _prep_in_maps_causal(x, inv_freqs, Wq, Wk, Wv, Wo, seq):
    import ml_dtypes
    bf = ml_dtypes.bfloat16
    scale = 1.0 / math.sqrt(D)
    perm = np.concatenate([np.arange(0, D, 2), np.arange(1, D, 2)])
    sign = np.where(np.arange(D) < D // 2, -1.0, 1.0)[:, None]
    ifT = np.ascontiguousarray(inv_freqs.reshape(seq, D).T)
    sinT = np.ascontiguousarray(np.sin(ifT)[perm] * sign).astype(bf)
    cosT = np.ascontiguousarray(np.cos(ifT)[perm]).astype(bf)
    cmask = _canonical_mask().astype(bf)

    shard = []
    for g in range(2):
        wqT = np.ascontiguousarray(
            (Wq[g * 1024:(g + 1) * 1024, :] * scale).T).astype(bf)
        wqT = np.ascontiguousarray(
            wqT.reshape(2048, 8, 128)[:, :, perm].reshape(2048, 1024))
        # tiled layout: [m, p, kc, c] so each head's weights are one
        # contiguous 4KB-per-partition DMA
        wqt = np.ascontiguousarray(
            wqT.reshape(16, 128, 8, 128).transpose(2, 1, 0, 3)
            .reshape(8, 128, 16 * 128))
        wkT = np.ascontiguousarray(Wk[g * 256:(g + 1) * 256, :].T).astype(bf)
        wkT = np.ascontiguousarray(
            wkT.reshape(2048, 2, 128)[:, :, perm].reshape(2048, 256))
        wvT = np.ascontiguousarray(Wv[g * 256:(g + 1) * 256, :].T).astype(bf)
        woT = np.ascontiguousarray(Wo[:, g * 1024:(g + 1) * 1024].T).astype(bf)
        shard.append((wqt, wkT, wvT, woT))

    in_maps = []
    for b in range(B):
        xTb = np.ascontiguousarray(x[b].T).astype(bf)
        for g in range(2):
            wqt, wkT, wvT, woT = shard[g]
            in_maps.append({
                "xT": xTb, "wqt": wqt, "wkT": wkT, "wvT": wvT, "woT": woT,
                "cmask": cmask, "sinT": sinT, "cosT": cosT,
            })
    return in_maps


def _check_causal(mask, seq):
    """True if blocks strictly above the diagonal may be skipped (mask very
    negative -> exp underflows to 0) and blocks at/below the diagonal need
    no mask add (mask exactly 0)."""
    m = mask.reshape(seq, seq)
    iu = np.triu_indices(seq, k=1)
    il = np.tril_indices(seq, k=0)
    return bool((m[iu] <= -1e4).all() and (m[il] == 0.0).all())


def kernel(x, start_pos, inv_freqs, mask, Wq, Wk, Wv, Wo, bo, _trace=False):
    from concourse.bass_utils import run_bass_kernel_spmd

    x = np.asarray(x, dtype=np.float32)
    inv_freqs = np.asarray(inv_freqs, dtype=np.float32)
    mask = np.asarray(mask, dtype=np.float32)
    Wq = np.asarray(Wq, dtype=np.float32)
    Wk = np.asarray(Wk, dtype=np.float32)
    Wv = np.asarray(Wv, dtype=np.float32)
    Wo = np.asarray(Wo, dtype=np.float32)
    bo = np.asarray(bo, dtype=np.float32)

    seq = x.shape[1]
    causal = _check_causal(mask, seq)
    nc = _get_compiled(seq, causal)
    if causal:
        in_maps = _prep_in_maps_causal(x, inv_freqs, Wq, Wk, Wv, Wo, seq)
    else:
        in_maps = _prep_in_maps_generic(x, inv_freqs, mask, Wq, Wk, Wv, Wo,
                                        seq)

    res = run_bass_kernel_spmd(nc, in_maps, core_ids=list(range(8)),
                               trace=_trace)
    outs = [r["out"] for r in res.results]
    y = np.empty((B, seq, C), dtype=np.float32)
    for b in range(B):
        y[b] = outs[2 * b] + outs[2 * b + 1] + bo[None, :]
    if _trace:
        kernel._last_results = res
    return y



# revision 9
# speedup vs baseline: 1.1074x; 1.1074x over previous
"""GQA attention kernel v3 for Trainium2, 8 NeuronCores.

Problem: B=4, T=2048, C=2048, H=16 q-heads, KVH=4 kv-heads, D=128, RoPE,
causal mask, out-projection with bias.

Sharding: (batch b, q-head-group g) -> core 2*b+g. Each core handles one
batch and 8 q heads (= 2 kv heads). The out-projection partial sums of the
two head-groups of a batch are summed on the host (+ bias).

v3 changes vs v2:
  - Q/K/V and out-projections run in fp8e4 (e4m3) DoubleRow perf mode
    (0.5 PE cycles/row, 2x128 contraction per matmul) with a 3-term
    hi/lo error-compensated split: W ~ (Wh + Wl)/sw, x ~ (xh + xl)/sx
    with power-of-2 scales so all cross terms share one PSUM descale.
    This keeps ~bf16 accuracy at 0.75x the bf16 PE cost.
  - softmax denominator via "transposed-l" matmuls: per 128-col slice of
    each prob block, a [128,128]-stationary x [128,1]-ones matmul gives
    l^T in PSUM at ~1 PE row each (replaces the fat ones-matmuls and the
    whole DVE l-accumulation chain).
  - scores S and attn*V stay bf16 (128-deep contraction does not pair).

Per-core dataflow (feature-major "T" layouts):
  xT  [C, T] fp8 hi/lo  activations (scaled by 16)
  QT  [128, 8, T] bf16  q projections, partition = d (RoPE'd)
  KT  [128, 2, T] bf16  keys
  V   [128, NKT, 256]   values bf16, partition = token within 128-tile
  S^T [Tk, Tq]          scores transposed; exp on ACT; l^T via tiny
                        matmuls; normalization in O^T epilogue
  OT  [128, 8, 512]     fp8 hi/lo (x16), for DoubleRow out-projection
  out [T, C]            partial out-projection (fp32)
"""

import math

import numpy as np

B, T, C = 4, 2048, 2048
H, KVH, D = 16, 4, 128
HPC = 8      # q heads per core
KVPC = 2     # kv heads per core
P = 128

SX = 16.0        # x scale
SWQ = 8192.0     # Wq*(1/sqrt(D)) scale
SWKV = 1024.0    # Wk/Wv scale
SWO = 1024.0     # Wo scale
SOT = 16.0       # OT scale (folded into the ones tile: 1/SOT)

_compiled = {}


def _build_causal(seq=T):
    import concourse.bacc as bacc
    import concourse.mybir as mybir
    import concourse.tile as tile

    f32 = mybir.dt.float32
    bf16 = mybir.dt.bfloat16
    fp8 = mybir.dt.float8e4
    AF = mybir.ActivationFunctionType
    ALU = mybir.AluOpType
    DR = mybir.MatmulPerfMode.DoubleRow

    NKT = seq // P          # k tiles of 128
    NG = seq // 512         # q groups of 512
    KC = C // P             # 16 contraction chunks
    NP = KC // 2            # 8 contraction pair-chunks
    RPH = HPC // KVPC       # q heads per kv head

    nc = bacc.Bacc(None, target_bir_lowering=False)

    xhi = nc.dram_tensor("xhi", [C, seq], fp8, kind="ExternalInput")
    xlo = nc.dram_tensor("xlo", [C, seq], fp8, kind="ExternalInput")
    wqh = nc.dram_tensor("wqh", [HPC, P, KC * P], fp8, kind="ExternalInput")
    wql = nc.dram_tensor("wql", [HPC, P, KC * P], fp8, kind="ExternalInput")
    wkh = nc.dram_tensor("wkh", [C, KVPC * D], fp8, kind="ExternalInput")
    wkl = nc.dram_tensor("wkl", [C, KVPC * D], fp8, kind="ExternalInput")
    wvh = nc.dram_tensor("wvh", [C, KVPC * D], fp8, kind="ExternalInput")
    wvl = nc.dram_tensor("wvl", [C, KVPC * D], fp8, kind="ExternalInput")
    woh = nc.dram_tensor("woh", [HPC * D, C], fp8, kind="ExternalInput")
    wol = nc.dram_tensor("wol", [HPC * D, C], fp8, kind="ExternalInput")
    cmask = nc.dram_tensor("cmask", [P, 4, 512], bf16, kind="ExternalInput")
    sinT = nc.dram_tensor("sinT", [D, seq], bf16, kind="ExternalInput")
    cosT = nc.dram_tensor("cosT", [D, seq], bf16, kind="ExternalInput")
    out = nc.dram_tensor("out", [seq, C], f32, kind="ExternalOutput")

    xhi_r = xhi.rearrange("(kc p) t -> p kc t", p=P)
    xlo_r = xlo.rearrange("(kc p) t -> p kc t", p=P)
    wkh_r = wkh.rearrange("(kc p) m -> p kc m", p=P)
    wkl_r = wkl.rearrange("(kc p) m -> p kc m", p=P)
    wvh_r = wvh.rearrange("(kc p) m -> p kc m", p=P)
    wvl_r = wvl.rearrange("(kc p) m -> p kc m", p=P)
    woh_r = woh.rearrange("(h p) c -> p h c", p=P)
    wol_r = wol.rearrange("(h p) c -> p h c", p=P)

    DSQ = 1.0 / (SX * SWQ)      # Q psum descale
    DSKV = 1.0 / (SX * SWKV)    # K/V psum descale
    DSO = 1.0 / (SOT * SWO)     # out-proj psum descale

    with (
        tile.TileContext(nc) as tc,
        tc.tile_pool(name="persist", bufs=1) as persist,
        tc.tile_pool(name="xh", bufs=2) as xhp,
    ):
        QT = persist.tile([P, HPC, seq], bf16)
        KT = persist.tile([P, KVPC, seq], bf16)
        V = persist.tile([P, NKT, KVPC * D], bf16)
        sin_t = persist.tile([P, seq], bf16)
        cos_t = persist.tile([P, seq], bf16)
        cm = persist.tile([P, 4, 512], bf16)
        ones = persist.tile([P, 1], bf16)
        wk8h = persist.tile([P, KC, KVPC * D], fp8)
        wk8l = persist.tile([P, KC, KVPC * D], fp8)
        wv8h = persist.tile([P, KC, KVPC * D], fp8)
        wv8l = persist.tile([P, KC, KVPC * D], fp8)
        wo8h = persist.tile([P, HPC, C], fp8)
        wo8l = persist.tile([P, HPC, C], fp8)
        wk8 = (wk8h, wk8l)
        wv8 = (wv8h, wv8l)
        wo8 = (wo8h, wo8l)

        # l^T is accumulated at 1/SOT so the reciprocal yields SOT/l and
        # the epilogue multiply lands OT pre-scaled for fp8
        nc.vector.memset(ones[:], 1.0 / SOT)

        # ======== phase 1: projections + RoPE ========
        NQ = seq // 512
        DEFER_V = set(range(max(1, NQ - 2), NQ))
        xq_tiles = {}

        def qkv_mms(ps, whi, wlo, xh_, xl_, msl):
            """3-term compensated DoubleRow accumulation into ps.
            whi/wlo: [P, KC, *] weight tiles (stationary, free slice msl);
            xh_/xl_: [P, KC, 512] x tiles (moving)."""
            terms = ((whi, xh_), (whi, xl_), (wlo, xh_))
            for ti, (wa, xa) in enumerate(terms):
                for jp in range(NP):
                    s2 = slice(2 * jp, 2 * jp + 2)
                    nc.tensor.matmul(
                        ps[:], wa[:, s2, msl], xa[:, s2, :],
                        start=(ti == 0 and jp == 0),
                        stop=(ti == 2 and jp == NP - 1),
                        perf_mode=DR)

        with (
            tc.tile_pool(name="wm", bufs=5) as wmp,
            tc.tile_pool(name="qrp", bufs=4) as qrp,
            tc.tile_pool(name="qsp", bufs=3) as qsp,
            tc.tile_pool(name="rtp", bufs=3) as rtp,
            tc.tile_pool(name="psq", bufs=4, space="PSUM") as psqp,
            tc.tile_pool(name="psv", bufs=2, space="PSUM") as psvp,
        ):
            for hf in range(NQ):
                gch = slice(hf * 512, (hf + 1) * 512)
                xqh = xhp.tile([P, KC, 512], fp8, tag="xqh")
                xql = xhp.tile([P, KC, 512], fp8, tag="xql")
                xq_tiles[hf] = (xqh, xql)

                def dma_xq():
                    for kc0 in range(0, KC, 4):
                        nc.sync.dma_start(
                            xqh[:, kc0:kc0 + 4, :],
                            xhi_r[:, kc0:kc0 + 4, gch])
                    for kc0 in range(0, KC, 4):
                        nc.sync.dma_start(
                            xql[:, kc0:kc0 + 4, :],
                            xlo_r[:, kc0:kc0 + 4, gch])

                def dma_wq(m):
                    wah = wmp.tile([P, KC, P], fp8, tag="wmh")
                    nc.sync.dma_start(
                        wah[:], wqh[m].rearrange("p (kc c) -> p kc c", c=P))
                    wal = wmp.tile([P, KC, P], fp8, tag="wml")
                    nc.sync.dma_start(
                        wal[:], wql[m].rearrange("p (kc c) -> p kc c", c=P))
                    return (wah, wal)

                if hf > 0:
                    # out-projection weights: stream on the ACT ring, gated
                    # behind phase-1 ACT work so they stay clear of the
                    # phase-1 SP DMA burst
                    for hh in range((hf - 1) * HPC // NQ, hf * HPC // NQ):
                        nc.scalar.dma_start(
                            wo8[0][:, hh:hh + 1, :], woh_r[:, hh:hh + 1, :])
                        nc.scalar.dma_start(
                            wo8[1][:, hh:hh + 1, :], wol_r[:, hh:hh + 1, :])
                if hf == 0:
                    # critical-path order: first weights, first x chunks,
                    # then tables and the resident K/V weights
                    wq_pre = {0: dma_wq(0)}
                    for kc0 in range(0, KC, 4):
                        nc.sync.dma_start(
                            xqh[:, kc0:kc0 + 4, :],
                            xhi_r[:, kc0:kc0 + 4, gch])
                        if kc0 == 0:
                            wq_pre[1] = dma_wq(1)
                        if kc0 == 8:
                            wq_pre[2] = dma_wq(2)
                    for kc0 in range(0, KC, 4):
                        nc.sync.dma_start(
                            xql[:, kc0:kc0 + 4, :],
                            xlo_r[:, kc0:kc0 + 4, gch])
                    wq_pre[3] = dma_wq(3)
                    wq_pre[4] = dma_wq(4)
                    nc.sync.dma_start(sin_t[:], sinT[:])
                    nc.sync.dma_start(cos_t[:], cosT[:])
                    # first two head chains: hh term column-major over kc
                    # pair groups so PE tracks the arriving xhi chunks, then
                    # the hl/lh terms once xlo lands
                    ps01 = []
                    for _i in range(2):
                        psq_cm = psqp.tile([P, 512], f32, tag="pq")
                        ps01.append(psq_cm)
                    for jp0 in range(0, NP, 2):
                        for m in range(2):
                            wah, _ = wq_pre[m]
                            for jp in range(jp0, jp0 + 2):
                                s2 = slice(2 * jp, 2 * jp + 2)
                                nc.tensor.matmul(
                                    ps01[m][:], wah[:, s2, :],
                                    xqh[:, s2, :], start=(jp == 0),
                                    stop=False, perf_mode=DR)
                    for m in range(2):
                        wah, wal = wq_pre[m]
                        for ti, wa_xa in enumerate(((wah, xql), (wal, xqh))):
                            wa, xa = wa_xa
                            for jp in range(NP):
                                s2 = slice(2 * jp, 2 * jp + 2)
                                nc.tensor.matmul(
                                    ps01[m][:], wa[:, s2, :], xa[:, s2, :],
                                    start=False,
                                    stop=(ti == 1 and jp == NP - 1),
                                    perf_mode=DR)
                    wq_head = {0: ps01[0], 1: ps01[1]}
                else:
                    dma_xq()
                    wq_pre = {}
                    wq_head = {}

                # Q (streamed weights) then K (resident) projections + RoPE
                for nm, ws, dstT, dsc in ((HPC, None, QT, DSQ),
                                          (KVPC, wk8, KT, DSKV)):
                    for m in range(nm):
                        if ws is None and hf == 0:
                            if m + 2 < HPC and (m + 2) not in wq_pre:
                                wq_pre[m + 2] = dma_wq(m + 2)
                            if m == HPC - 2:
                                nc.sync.dma_start(wk8[0][:], wkh_r[:])
                                nc.sync.dma_start(wk8[1][:], wkl_r[:])
                                nc.sync.dma_start(wv8[0][:], wvh_r[:])
                                nc.sync.dma_start(wv8[1][:], wvl_r[:])
                                nc.sync.dma_start(cm[:], cmask[:])
                        if ws is None and m in wq_head:
                            psq = wq_head[m]
                        else:
                            psq = psqp.tile([P, 512], f32, tag="pq")
                            if ws is None:
                                wah, wal = (wq_pre[m] if m in wq_pre
                                            else dma_wq(m))
                                qkv_mms(psq, wah, wal, xqh, xql,
                                        slice(None))
                            else:
                                qkv_mms(psq, ws[0], ws[1], xqh, xql,
                                        slice(m * P, (m + 1) * P))
                        qr = qrp.tile([P, 512], bf16, tag="qr")
                        nc.scalar.activation(qr[:], psq[:], AF.Copy,
                                             scale=dsc)
                        qs = qsp.tile([P, 512], bf16, tag="qs")
                        nc.scalar.dma_start(qs[0:64, :], qr[64:128, :])
                        nc.scalar.dma_start(qs[64:128, :], qr[0:64, :])
                        rt = rtp.tile([P, 512], bf16, tag="rt")
                        nc.vector.tensor_tensor(
                            rt[:], qs[:], sin_t[:, gch], ALU.mult)
                        nc.vector.tensor_tensor(
                            dstT[:, m, gch], qr[:], cos_t[:, gch], ALU.mult)
                        nc.vector.tensor_tensor(
                            dstT[:, m, gch], dstT[:, m, gch], rt[:], ALU.add)

                # V projection; the last chunks are deferred into the start
                # of the attention phase to fill its PE bubbles
                if hf not in DEFER_V:
                    for tt in range(4):
                        gtt = hf * 4 + tt
                        psv = psvp.tile([P, KVPC * D], f32, tag="pv")
                        tsl = slice(tt * P, (tt + 1) * P)
                        # stationary = x slice, moving = wv
                        for ti, (xa, wa) in enumerate(
                                ((xqh, wv8[0]), (xql, wv8[0]), (xqh, wv8[1]))):
                            for jp in range(NP):
                                s2 = slice(2 * jp, 2 * jp + 2)
                                nc.tensor.matmul(
                                    psv[:], xa[:, s2, tsl], wa[:, s2, :],
                                    start=(ti == 0 and jp == 0),
                                    stop=(ti == 2 and jp == NP - 1),
                                    perf_mode=DR)
                        nc.scalar.activation(V[:, gtt, :], psv[:], AF.Copy,
                                             scale=DSKV)

        start_h = (NQ - 1) * HPC // NQ if NQ > 1 else 0
        for hh in range(start_h, HPC):
            nc.scalar.dma_start(
                wo8[0][:, hh:hh + 1, :], woh_r[:, hh:hh + 1, :])
            nc.scalar.dma_start(
                wo8[1][:, hh:hh + 1, :], wol_r[:, hh:hh + 1, :])

        # ======== phases 2+3: attention + interleaved out-projection ====
        # Chunk order: groups 0 and 1 interleaved (g0 alone is too small to
        # keep PE busy), then groups 2, 3. Out-projection chains for group g
        # are injected into later chunks to fill exp-latency PE bubbles.
        with (
            tc.tile_pool(name="otp", bufs=3) as otp,
            tc.tile_pool(name="pch", bufs=7) as pch,
            tc.tile_pool(name="lsp", bufs=2) as lsp,
            tc.tile_pool(name="lbp", bufs=2) as lbp,
            tc.tile_pool(name="sm2", bufs=4) as sm2,
            tc.tile_pool(name="obp", bufs=3) as obp,
            tc.tile_pool(name="ps_s", bufs=3, space="PSUM") as ps_s,
            tc.tile_pool(name="ps_o", bufs=2, space="PSUM") as ps_o,
            tc.tile_pool(name="ps_l", bufs=1, space="PSUM") as ps_l,
            tc.tile_pool(name="ps3", bufs=2, space="PSUM") as ps3p,
        ):
            OTg = {}        # group -> (OT_hi, OT_lo) fp8 tiles [P, HPC, 512]
            pending = []    # deferred PE work (closures)

            def chain_mms(g, tt, co, ps3, tis):
                csl = slice(co * 512, (co + 1) * 512)
                tsl = slice(tt * P, (tt + 1) * P)
                oth, otl = OTg[g]
                terms = ((oth, wo8[0]), (oth, wo8[1]), (otl, wo8[0]))
                for ti in tis:
                    ota, wa = terms[ti]
                    for hp in range(HPC // 2):
                        s2 = slice(2 * hp, 2 * hp + 2)
                        nc.tensor.matmul(
                            ps3[:], ota[:, s2, tsl], wa[:, s2, csl],
                            start=(ti == 0 and hp == 0),
                            stop=(ti == 2 and hp == HPC // 2 - 1),
                            perf_mode=DR)

            def chain_evac(g, tt, co, ps3):
                csl = slice(co * 512, (co + 1) * 512)
                ob = obp.tile([P, 512], f32, tag="ob")
                nc.scalar.activation(ob[:], ps3[:], AF.Copy, scale=DSO)
                nc.sync.dma_start(
                    out[(4 * g + tt) * P:(4 * g + tt + 1) * P, csl], ob[:])

            def emit_chain(g, tt, co):
                """One out-projection chain: 12 DR matmuls + evac + DMA."""
                ps3 = ps3p.tile([P, 512], f32, tag="po")
                chain_mms(g, tt, co, ps3, (0, 1, 2))
                chain_evac(g, tt, co, ps3)

            def emit_vchain(hf, tt):
                """Deferred V projection for one 128-token tile."""
                xqh, xql = xq_tiles[hf]
                gtt = hf * 4 + tt
                tsl = slice(tt * P, (tt + 1) * P)
                psv = ps3p.tile([P, KVPC * D], f32, tag="po")
                for ti, (xa, wa) in enumerate(
                        ((xqh, wv8[0]), (xql, wv8[0]), (xqh, wv8[1]))):
                    for jp in range(NP):
                        s2 = slice(2 * jp, 2 * jp + 2)
                        nc.tensor.matmul(
                            psv[:], xa[:, s2, tsl], wa[:, s2, :],
                            start=(ti == 0 and jp == 0),
                            stop=(ti == 2 and jp == NP - 1),
                            perf_mode=DR)
                nc.scalar.activation(V[:, gtt, :], psv[:], AF.Copy,
                                     scale=DSKV)

            def drain(k):
                for _ in range(min(k, len(pending))):
                    kind, args = pending.pop(0)
                    if kind == "v":
                        emit_vchain(*args)
                    else:
                        emit_chain(*args)

            for dhf in sorted(DEFER_V):
                for tt in range(4):
                    pending.append(("v", (dhf, tt)))

            order = [(g, h) for g in range(NG) for h in range(HPC)]

            class Chunk:
                def __init__(self, g, h):
                    self.g, self.h = g, h
                    self.kv = h // RPH
                    self.ntk = 4 * (g + 1)
                    self.pso = ps_o.tile([P, 512], f32, tag="o")
                    self.psl = ps_l.tile([P, 4], f32, tag="l")

                def q0_of(self, j):
                    return 128 * max(0, j - 4 * self.g)

                def emit_s(self, j):
                    q0 = self.q0_of(j)
                    pss = ps_s.tile([P, 512], f32, tag="s")
                    nc.tensor.matmul(
                        pss[:, q0:], KT[:, self.kv, j * P:(j + 1) * P],
                        QT[:, self.h, self.g * 512 + q0:(self.g + 1) * 512],
                        start=True, stop=True)
                    u = j - 4 * self.g
                    if u >= 0:
                        # mask is nontrivial only on the diagonal 128 cols
                        nc.vector.tensor_tensor(
                            pss[:, q0:q0 + P], pss[:, q0:q0 + P],
                            cm[:, u, q0:q0 + P], ALU.add)
                    return pss

                def emit_lo(self, i, pss):
                    """exp, O-matmul, and transposed-l matmuls for block i."""
                    g, ntk = self.g, self.ntk
                    q0 = self.q0_of(i)
                    u = i - 4 * g
                    pc = pch.tile([P, 512], bf16, tag="p")
                    nc.scalar.activation(pc[:, q0:], pss[:, q0:], AF.Exp)
                    nc.tensor.matmul(
                        self.pso[:, q0:],
                        V[:, i, self.kv * D:(self.kv + 1) * D],
                        pc[:, q0:], start=(i == 0), stop=(i == ntk - 1))
                    # l^T: one tiny matmul per valid 128-col slice; column j
                    # accumulates over blocks i=0..4g+j
                    j0 = max(0, u)
                    for j in range(j0, 4):
                        nc.tensor.matmul(
                            self.psl[:, j:j + 1],
                            pc[:, j * P:(j + 1) * P], ones[:],
                            start=(i == 0), stop=(i == 4 * g + j))

                def epilogue(self):
                    # psl holds l/SOT transposed [q-in-slice, slice]:
                    # reciprocal -> SOT/l while still transposed (4 elems),
                    # DMA to row layout, broadcast, normalize, and split OT
                    # into fp8 hi/lo for the DoubleRow chains
                    lr4 = lsp.tile([P, 4], bf16, tag="lr4")
                    with nc.allow_low_precision(
                            reason="softmax denom reciprocal in bf16"):
                        nc.vector.reciprocal(lr4[:], self.psl[:])
                    lrc = sm2.tile([1, 4, P], bf16, tag="lrc")
                    for j in range(4):
                        nc.gpsimd.dma_start(lrc[0:1, j, :], lr4[:, j:j + 1])
                    lb = lbp.tile([P, 512], bf16, tag="lb")
                    nc.gpsimd.partition_broadcast(
                        lb[:], lrc[:].rearrange("a j p -> a (j p)"))
                    oth, otl = OTg[self.g]
                    obt = sm2.tile([P, 512], bf16, tag="obt")
                    nc.vector.tensor_tensor(
                        obt[:], self.pso[:], lb[:], ALU.mult)
                    with nc.allow_low_precision(
                            reason="fp8 hi/lo split of OT"):
                        nc.vector.tensor_copy(oth[:, self.h, :], obt[:])
                        nc.vector.tensor_tensor(
                            otl[:, self.h, :], obt[:], oth[:, self.h, :],
                            ALU.subtract)

            # flat block stream with cross-chunk S-prefetch (depth PF)
            PF = 2
            for g, h in order:
                if g not in OTg:
                    ot_gh = otp.tile([P, HPC, 512], fp8, tag="oth")
                    ot_gl = otp.tile([P, HPC, 512], fp8, tag="otl")
                    OTg[g] = (ot_gh, ot_gl)

            flat = []        # (chunk key, local block index)
            for g, h in order:
                for j in range(4 * (g + 1)):
                    flat.append(((g, h), j))

            done_heads = {g: 0 for g in range(NG)}
            live = {}
            spos = 0

            def ensure_chunk(key):
                if key not in live:
                    live[key] = Chunk(*key)
                return live[key]

            def emit_s_at(idx):
                key, j = flat[idx]
                ck = ensure_chunk(key)
                return ck, ck.emit_s(j)

            squeue = []
            for _ in range(min(PF, len(flat))):
                squeue.append(emit_s_at(spos))
                spos += 1
            drain(1)
            for idx in range(len(flat)):
                key, j = flat[idx]
                ck, pss = squeue.pop(0)
                ck.emit_lo(j, pss)
                if spos < len(flat):
                    squeue.append(emit_s_at(spos))
                    spos += 1
                if j == ck.ntk - 1:
                    ck.epilogue()
                    del live[key]
                    g = key[0]
                    done_heads[g] += 1
                    if done_heads[g] == HPC:
                        for tt in range(4):
                            for co in range(C // 512):
                                pending.append(("c", (g, tt, co)))
                    drain(1)
                elif j == ck.ntk - 3:
                    # keep chains in reserve so the last chunks' exp tails
                    # still have PE fill work
                    if idx >= len(flat) - 2 * ck.ntk:
                        drain(2)
                    elif len(pending) > 5:
                        drain(1)
            if len(pending) >= 2 and pending[0][0] == "c" \
                    and pending[1][0] == "c":
                a = pending.pop(0)[1]
                b = pending.pop(0)[1]
                psa = ps3p.tile([P, 512], f32, tag="po")
                psb = ps3p.tile([P, 512], f32, tag="po")
                chain_mms(*a, psa, (0, 1))
                chain_mms(*b, psb, (0, 1))
                chain_mms(*a, psa, (2,))
                chain_evac(*a, psa)
                chain_mms(*b, psb, (2,))
                chain_evac(*b, psb)
            drain(len(pending))

    nc.finalize()
    return nc


def _build_generic(seq=T, causal=False):
    import concourse.bacc as bacc
    import concourse.mybir as mybir
    import concourse.tile as tile

    f32 = mybir.dt.float32
    f32r = mybir.dt.float32r
    i32 = mybir.dt.int32
    AF = mybir.ActivationFunctionType
    ALU = mybir.AluOpType

    NKT = seq // P          # Tk tiles of 128
    NG = seq // 512         # Tq groups of 512
    KC = C // P             # 16 contraction chunks
    RPH = HPC // KVPC       # q heads per kv head

    nc = bacc.Bacc(None, target_bir_lowering=False)

    xT = nc.dram_tensor("xT", [C, seq], f32r, kind="ExternalInput")
    wqT = nc.dram_tensor("wqT", [C, HPC * D], f32r, kind="ExternalInput")
    wkT = nc.dram_tensor("wkT", [C, KVPC * D], f32r, kind="ExternalInput")
    wvT = nc.dram_tensor("wvT", [C, KVPC * D], f32r, kind="ExternalInput")
    woT = nc.dram_tensor("woT", [HPC * D, C], f32r, kind="ExternalInput")
    maskT = nc.dram_tensor("maskT", [seq, seq], f32, kind="ExternalInput")
    ifT = nc.dram_tensor("ifT", [D, seq], f32, kind="ExternalInput")
    rmat = nc.dram_tensor("rmat", [D, D], f32r, kind="ExternalInput")
    out = nc.dram_tensor("out", [seq, C], f32, kind="ExternalOutput")

    xT_r = xT.rearrange("(kc p) t -> p kc t", p=P)
    wqT_r = wqT.rearrange("(kc p) m -> p kc m", p=P)
    wkT_r = wkT.rearrange("(kc p) m -> p kc m", p=P)
    wvT_r = wvT.rearrange("(kc p) m -> p kc m", p=P)
    woT_r = woT.rearrange("(h p) c -> p h c", p=P)

    with (
        tile.TileContext(nc) as tc,
        tc.tile_pool(name="persist", bufs=1) as persist,
        tc.tile_pool(name="small", bufs=4) as small,
    ):
        QT = persist.tile([P, HPC, seq], f32r)
        KT = persist.tile([P, KVPC, seq], f32r)
        V = persist.tile([P, NKT, KVPC * D], f32r)
        rm = persist.tile([P, D], f32r)
        nc.sync.dma_start(rm[:], rmat[:])
        ones32 = small.tile([P, 1], f32)
        nc.vector.memset(ones32[:], 1.0)
        ones = persist.tile([P, 1], f32r)
        nc.vector.tensor_copy(ones[:], ones32[:])

        # ======== phase 1: trig tables, projections, RoPE ========
        with tc.tile_pool(name="ph1", bufs=1) as ph1:
            sinT = ph1.tile([P, seq], f32)
            cosT = ph1.tile([P, seq], f32)
            wv_sb = ph1.tile([P, KC, KVPC * D], f32r)

            # sin/cos tables via range-reduced LUT sin:
            # f = (theta/2pi + shift) mod 1;  sin(2*pi*f)
            with tc.tile_pool(name="trig", bufs=1) as trig:
                tf_ = trig.tile([P, seq], f32, tag="tf")
                nc.sync.dma_start(tf_[:], ifT[:])
                inv2pi = float(1.0 / (2.0 * math.pi))
                for dst, shift in ((sinT, 0.0), (cosT, 0.25)):
                    ty = trig.tile([P, seq], f32, tag="ty")
                    nc.vector.tensor_scalar_mul(ty[:], tf_[:], inv2pi)
                    if shift:
                        nc.vector.tensor_scalar_add(ty[:], ty[:], shift)
                    ti_ = trig.tile([P, seq], i32, tag="ti")
                    nc.vector.tensor_copy(ti_[:], ty[:])
                    tfr = trig.tile([P, seq], f32, tag="tfr")
                    nc.vector.tensor_copy(tfr[:], ti_[:])
                    nc.vector.tensor_tensor(ty[:], ty[:], tfr[:], ALU.subtract)
                    nc.scalar.activation(dst[:], ty[:], AF.Sin,
                                         scale=float(2.0 * math.pi))

            with (
                tc.tile_pool(name="xh", bufs=2) as xhp,
                tc.tile_pool(name="wm", bufs=3) as wmp,
                tc.tile_pool(name="praw", bufs=1) as praw,
                tc.tile_pool(name="ps1", bufs=2, space="PSUM") as ps1,
            ):
                NQ = seq // 512
                for hf in range(NQ):
                    gch = slice(hf * 512, (hf + 1) * 512)
                    xq = xhp.tile([P, KC, 512], f32r, tag="xq")
                    nc.sync.dma_start(xq[:], xT_r[:, :, gch])
                    if hf == 0:
                        nc.sync.dma_start(wv_sb[:], wvT_r[:])

                    # Q then K projections + RoPE
                    for nm, wr, dstT in ((HPC, wqT_r, QT), (KVPC, wkT_r, KT)):
                        for m in range(nm):
                            wa = wmp.tile([P, KC // 2, P], f32r, tag="wm")
                            nc.sync.dma_start(
                                wa[:], wr[:, :KC // 2, m * P:(m + 1) * P])
                            wb = wmp.tile([P, KC // 2, P], f32r, tag="wm")
                            nc.sync.dma_start(
                                wb[:], wr[:, KC // 2:, m * P:(m + 1) * P])
                            psq = ps1.tile([P, 512], f32, tag="pq")
                            for kc in range(KC):
                                wt = wa if kc < KC // 2 else wb
                                nc.tensor.matmul(
                                    psq[:], wt[:, kc % (KC // 2), :],
                                    xq[:, kc, :],
                                    start=(kc == 0), stop=(kc == KC - 1))
                            qr = praw.tile([P, 512], f32r, tag="qr")
                            nc.scalar.activation(qr[:], psq[:], AF.Copy)
                            psr = ps1.tile([P, 512], f32, tag="pr")
                            nc.tensor.matmul(psr[:], rm[:], qr[:],
                                             start=True, stop=True)
                            # dst = qr*cos + rot*sin (rot*sin in-place)
                            nc.vector.tensor_tensor(
                                psr[:], psr[:], sinT[:, gch], ALU.mult)
                            nc.vector.tensor_tensor(
                                dstT[:, m, gch], qr[:].bitcast(f32),
                                cosT[:, gch], ALU.mult)
                            nc.vector.tensor_tensor(
                                dstT[:, m, gch],
                                dstT[:, m, gch].bitcast(f32), psr[:],
                                ALU.add)

                    # V projection
                    for tt in range(4):
                        gtt = hf * 4 + tt
                        psv = ps1.tile([P, KVPC * D], f32, tag="pv")
                        for kc in range(KC):
                            nc.tensor.matmul(
                                psv[:], xq[:, kc, tt * P:(tt + 1) * P],
                                wv_sb[:, kc, :],
                                start=(kc == 0), stop=(kc == KC - 1))
                        nc.scalar.activation(V[:, gtt, :], psv[:], AF.Copy)

        # ======== phases 2+3 ========
        with tc.tile_pool(name="otp", bufs=1) as otp:
            OT = otp.tile([P, HPC, seq], f32r)

            with (
                tc.tile_pool(name="mb", bufs=2) as mbp,
                tc.tile_pool(name="pch", bufs=3) as pch,
                tc.tile_pool(name="lbp", bufs=2) as lbp,
                tc.tile_pool(name="sm2", bufs=4) as sm2,
                tc.tile_pool(name="ps_s", bufs=4, space="PSUM") as ps_s,
                tc.tile_pool(name="ps_o", bufs=2, space="PSUM") as ps_o,
                tc.tile_pool(name="ps_l", bufs=2, space="PSUM") as ps_l,
            ):
                for g in range(NG):
                    qsl = slice(g * 512, (g + 1) * 512)
                    nb = 4 if causal else NKT
                    i0 = 4 * g if causal else 0
                    ntk = 4 * (g + 1) if causal else NKT
                    mb = mbp.tile([P, nb, 512], f32, tag="mb")
                    nc.sync.dma_start(
                        mb[:],
                        maskT[i0 * P:(i0 + nb) * P, qsl].rearrange(
                            "(i p) t -> p i t", p=P))
                    for h in range(HPC):
                        kv = h // RPH
                        pso = ps_o.tile([P, 512], f32, tag="o")
                        psl = ps_l.tile([1, 512], f32, tag="l")

                        def emit_s(j):
                            pss = ps_s.tile([P, 512], f32, tag="s")
                            nc.tensor.matmul(
                                pss[:], KT[:, kv, j * P:(j + 1) * P],
                                QT[:, h, qsl], start=True, stop=True)
                            if j >= i0:
                                nc.vector.tensor_tensor(
                                    pss[:], pss[:], mb[:, j - i0, :], ALU.add)
                            return pss

                        sq = [emit_s(j) for j in range(min(2, ntk))]
                        for i in range(ntk):
                            pss = sq.pop(0)
                            pc = pch.tile([P, 512], f32r, tag="p")
                            nc.scalar.activation(pc[:], pss[:], AF.Exp)
                            if i + 2 < ntk:
                                sq.append(emit_s(i + 2))
                            nc.tensor.matmul(
                                psl[:], ones[:], pc[:],
                                start=(i == 0), stop=(i == ntk - 1))
                            nc.tensor.matmul(
                                pso[:], V[:, i, kv * D:(kv + 1) * D], pc[:],
                                start=(i == 0), stop=(i == ntk - 1))
                        lsb = sm2.tile([1, 512], f32, tag="lsb")
                        nc.vector.tensor_copy(lsb[:], psl[:])
                        lrc = sm2.tile([1, 512], f32, tag="lrc")
                        nc.vector.reciprocal(lrc[:], lsb[:])
                        lb = lbp.tile([P, 512], f32, tag="lb")
                        nc.gpsimd.partition_broadcast(lb[:], lrc[:])
                        nc.vector.tensor_tensor(
                            OT[:, h, qsl], pso[:], lb[:], ALU.mult)

            # out-projection, co-chunk outer so weight slices stream once
            with (
                tc.tile_pool(name="wo", bufs=12) as wop,
                tc.tile_pool(name="ob", bufs=3) as obp,
                tc.tile_pool(name="ps3", bufs=4, space="PSUM") as ps3,
            ):
                for co in range(C // 512):
                    csl = slice(co * 512, (co + 1) * 512)
                    woh = []
                    for h in range(HPC):
                        w = wop.tile([P, 512], f32r, tag="wo")
                        nc.sync.dma_start(w[:], woT_r[:, h, csl])
                        woh.append(w)
                    for tt in range(NKT):
                        pso3 = ps3.tile([P, 512], f32, tag="po")
                        for h in range(HPC):
                            nc.tensor.matmul(
                                pso3[:], OT[:, h, tt * P:(tt + 1) * P],
                                woh[h][:],
                                start=(h == 0), stop=(h == HPC - 1))
                        ob = obp.tile([P, 512], f32, tag="ob")
                        nc.scalar.activation(ob[:], pso3[:], AF.Copy)
                        nc.sync.dma_start(
                            out[tt * P:(tt + 1) * P, csl], ob[:])

    nc.finalize()
    return nc


def _prep_in_maps_generic(x, inv_freqs, mask, Wq, Wk, Wv, Wo, seq):
    scale = 1.0 / math.sqrt(D)
    maskT = np.ascontiguousarray(mask.reshape(seq, seq).T)
    ifT = np.ascontiguousarray(inv_freqs.reshape(seq, D).T)
    rmat = _rope_rmat()

    shard = []
    for g in range(2):
        wqT = np.ascontiguousarray((Wq[g * 1024:(g + 1) * 1024, :] * scale).T)
        wkT = np.ascontiguousarray(Wk[g * 256:(g + 1) * 256, :].T)
        wvT = np.ascontiguousarray(Wv[g * 256:(g + 1) * 256, :].T)
        woT = np.ascontiguousarray(Wo[:, g * 1024:(g + 1) * 1024].T)
        shard.append((wqT, wkT, wvT, woT))

    in_maps = []
    for b in range(B):
        xTb = np.ascontiguousarray(x[b].T)
        for g in range(2):
            wqT, wkT, wvT, woT = shard[g]
            in_maps.append({
                "xT": xTb, "wqT": wqT, "wkT": wkT, "wvT": wvT, "woT": woT,
                "maskT": maskT, "ifT": ifT, "rmat": rmat,
            })
    return in_maps


def _get_compiled(seq, causal):
    key = (seq, causal)
    if key not in _compiled:
        if causal:
            _compiled[key] = _build_causal(seq)
        else:
            _compiled[key] = _build_generic(seq)
    return _compiled[key]


def _rope_rmat():
    # lhsT for rot = Pmat @ q, Pmat[2i, 2i+1] = -1, Pmat[2i+1, 2i] = 1:
    # lhsT[d', d] = Pmat[d, d']
    m = np.zeros((D, D), dtype=np.float32)
    for i in range(D // 2):
        m[2 * i + 1, 2 * i] = -1.0
        m[2 * i, 2 * i + 1] = 1.0
    return m


def _canonical_mask():
    # cmask[p, u, t] = 0 if 128u + p <= t else -1e9, for t in [0, 512)
    p = np.arange(P)[:, None, None]
    u = np.arange(4)[None, :, None]
    t = np.arange(512)[None, None, :]
    return np.where(128 * u + p <= t, 0.0, -1e9).astype(np.float32)


def _split8(a, s):
    """hi/lo fp8e4 split of a*s (s a power of 2). Returns (hi, lo)."""
    import ml_dtypes
    f8 = ml_dtypes.float8_e4m3
    a = np.asarray(a, dtype=np.float32) * np.float32(s)
    hi = a.astype(f8)
    lo = (a - hi.astype(np.float32)).astype(f8)
    return hi, lo


def _prep_in_maps_causal(x, inv_freqs, Wq, Wk, Wv, Wo, seq):
    import ml_dtypes
    bf = ml_dtypes.bfloat16
    scale = 1.0 / math.sqrt(D)
    perm = np.concatenate([np.arange(0, D, 2), np.arange(1, D, 2)])
    sign = np.where(np.arange(D) < D // 2, -1.0, 1.0)[:, None]
    ifT = np.ascontiguousarray(inv_freqs.reshape(seq, D).T)
    sinT = np.ascontiguousarray(np.sin(ifT)[perm] * sign).astype(bf)
    cosT = np.ascontiguousarray(np.cos(ifT)[perm]).astype(bf)
    cmask = _canonical_mask().astype(bf)

    shard = []
    for g in range(2):
        wqT = np.ascontiguousarray((Wq[g * 1024:(g + 1) * 1024, :] * scale).T)
        wqT = np.ascontiguousarray(
            wqT.reshape(2048, 8, 128)[:, :, perm].reshape(2048, 1024))
        # tiled layout: [m, p, kc, c] so each head's weights are one
        # contiguous DMA
        wqt = np.ascontiguousarray(
            wqT.reshape(16, 128, 8, 128).transpose(2, 1, 0, 3)
            .reshape(8, 128, 16 * 128))
        wqt_h, wqt_l = _split8(wqt, SWQ)
        wkT = np.ascontiguousarray(Wk[g * 256:(g + 1) * 256, :].T)
        wkT = np.ascontiguousarray(
            wkT.reshape(2048, 2, 128)[:, :, perm].reshape(2048, 256))
        wk_h, wk_l = _split8(wkT, SWKV)
        wvT = np.ascontiguousarray(Wv[g * 256:(g + 1) * 256, :].T)
        wv_h, wv_l = _split8(wvT, SWKV)
        woT = np.ascontiguousarray(Wo[:, g * 1024:(g + 1) * 1024].T)
        wo_h, wo_l = _split8(woT, SWO)
        shard.append((wqt_h, wqt_l, wk_h, wk_l, wv_h, wv_l, wo_h, wo_l))

    in_maps = []
    for b in range(B):
        xTb = np.ascontiguousarray(x[b].T)
        x_h, x_l = _split8(xTb, SX)
        for g in range(2):
            (wqt_h, wqt_l, wk_h, wk_l, wv_h, wv_l, wo_h, wo_l) = shard[g]
            in_maps.append({
                "xhi": x_h, "xlo": x_l,
                "wqh": wqt_h, "wql": wqt_l,
                "wkh": wk_h, "wkl": wk_l,
                "wvh": wv_h, "wvl": wv_l,
                "woh": wo_h, "wol": wo_l,
                "cmask": cmask, "sinT": sinT, "cosT": cosT,
            })
    return in_maps


def _check_causal(mask, seq):
    """True if blocks strictly above the diagonal may be skipped (mask very
    negative -> exp underflows to 0) and blocks at/below the diagonal need
    no mask add (mask exactly 0)."""
    m = mask.reshape(seq, seq)
    iu = np.triu_indices(seq, k=1)
    il = np.tril_indices(seq, k=0)
    return bool((m[iu] <= -1e4).all() and (m[il] == 0.0).all())


def kernel(x, start_pos, inv_freqs, mask, Wq, Wk, Wv, Wo, bo, _trace=False):
    from concourse.bass_utils import run_bass_kernel_spmd

    x = np.asarray(x, dtype=np.float32)
    inv_freqs = np.asarray(inv_freqs, dtype=np.float32)
    mask = np.asarray(mask, dtype=np.float32)
    Wq = np.asarray(Wq, dtype=np.float32)
    Wk = np.asarray(Wk, dtype=np.float32)
    Wv = np.asarray(Wv, dtype=np.float32)
    Wo = np.asarray(Wo, dtype=np.float32)
    bo = np.asarray(bo, dtype=np.float32)

    seq = x.shape[1]
    causal = _check_causal(mask, seq)
    nc = _get_compiled(seq, causal)
    if causal:
        in_maps = _prep_in_maps_causal(x, inv_freqs, Wq, Wk, Wv, Wo, seq)
    else:
        in_maps = _prep_in_maps_generic(x, inv_freqs, mask, Wq, Wk, Wv, Wo,
                                        seq)

    res = run_bass_kernel_spmd(nc, in_maps, core_ids=list(range(8)),
                               trace=_trace)
    outs = [r["out"] for r in res.results]
    y = np.empty((B, seq, C), dtype=np.float32)
    for b in range(B):
        y[b] = outs[2 * b] + outs[2 * b + 1] + bo[None, :]
    if _trace:
        kernel._last_results = res
    return y


# revision 12
# speedup vs baseline: 1.1500x; 1.0385x over previous
"""GQA attention kernel v3 for Trainium2, 8 NeuronCores.

Problem: B=4, T=2048, C=2048, H=16 q-heads, KVH=4 kv-heads, D=128, RoPE,
causal mask, out-projection with bias.

Sharding: (batch b, q-head-group g) -> core 2*b+g. Each core handles one
batch and 8 q heads (= 2 kv heads). The out-projection partial sums of the
two head-groups of a batch are summed on the host (+ bias).

v3 changes vs v2:
  - Q/K/V and out-projections run in fp8e4 (e4m3) DoubleRow perf mode
    (0.5 PE cycles/row, 2x128 contraction per matmul) with a 3-term
    hi/lo error-compensated split: W ~ (Wh + Wl)/sw, x ~ (xh + xl)/sx
    with power-of-2 scales so all cross terms share one PSUM descale.
    This keeps ~bf16 accuracy at 0.75x the bf16 PE cost.
  - softmax denominator via "transposed-l" matmuls: per 128-col slice of
    each prob block, a [128,128]-stationary x [128,1]-ones matmul gives
    l^T in PSUM at ~1 PE row each (replaces the fat ones-matmuls and the
    whole DVE l-accumulation chain).
  - scores S and attn*V stay bf16 (128-deep contraction does not pair).

Per-core dataflow (feature-major "T" layouts):
  xT  [C, T] fp8 hi/lo  activations (scaled by 16)
  QT  [128, 8, T] bf16  q projections, partition = d (RoPE'd)
  KT  [128, 2, T] bf16  keys
  V   [128, NKT, 256]   values bf16, partition = token within 128-tile
  S^T [Tk, Tq]          scores transposed; exp on ACT; l^T via tiny
                        matmuls; normalization in O^T epilogue
  OT  [128, 8, 512]     fp8 hi/lo (x16), for DoubleRow out-projection
  out [T, C]            partial out-projection (fp32)
"""

import math

import numpy as np

B, T, C = 4, 2048, 2048
H, KVH, D = 16, 4, 128
HPC = 8      # q heads per core
KVPC = 2     # kv heads per core
P = 128

SX = 16.0        # x scale
SWQ = 8192.0     # Wq*(1/sqrt(D)) scale
SWKV = 1024.0    # Wk/Wv scale
SWO = 1024.0     # Wo scale
SOT = 16.0       # OT scale (folded into the ones tile: 1/SOT)

_compiled = {}


def _build_causal(seq=T):
    import concourse.bacc as bacc
    import concourse.mybir as mybir
    import concourse.tile as tile

    f32 = mybir.dt.float32
    bf16 = mybir.dt.bfloat16
    fp8 = mybir.dt.float8e4
    AF = mybir.ActivationFunctionType
    ALU = mybir.AluOpType
    DR = mybir.MatmulPerfMode.DoubleRow

    NKT = seq // P          # k tiles of 128
    NG = seq // 512         # q groups of 512
    KC = C // P             # 16 contraction chunks
    NP = KC // 2            # 8 contraction pair-chunks
    RPH = HPC // KVPC       # q heads per kv head

    nc = bacc.Bacc(None, target_bir_lowering=False)

    xhi = nc.dram_tensor("xhi", [C, seq], fp8, kind="ExternalInput")
    xlo = nc.dram_tensor("xlo", [C, seq], fp8, kind="ExternalInput")
    wqh = nc.dram_tensor("wqh", [HPC, P, KC * P], fp8, kind="ExternalInput")
    wql = nc.dram_tensor("wql", [HPC, P, KC * P], fp8, kind="ExternalInput")
    wkh = nc.dram_tensor("wkh", [C, KVPC * D], fp8, kind="ExternalInput")
    wkl = nc.dram_tensor("wkl", [C, KVPC * D], fp8, kind="ExternalInput")
    wvh = nc.dram_tensor("wvh", [C, KVPC * D], fp8, kind="ExternalInput")
    wvl = nc.dram_tensor("wvl", [C, KVPC * D], fp8, kind="ExternalInput")
    woh = nc.dram_tensor("woh", [HPC * D, C], fp8, kind="ExternalInput")
    wol = nc.dram_tensor("wol", [HPC * D, C], fp8, kind="ExternalInput")
    cmask = nc.dram_tensor("cmask", [P, 4, 512], bf16, kind="ExternalInput")
    sinT = nc.dram_tensor("sinT", [D, seq], bf16, kind="ExternalInput")
    cosT = nc.dram_tensor("cosT", [D, seq], bf16, kind="ExternalInput")
    out = nc.dram_tensor("out", [seq, C], f32, kind="ExternalOutput")

    xhi_r = xhi.rearrange("(kc p) t -> p kc t", p=P)
    xlo_r = xlo.rearrange("(kc p) t -> p kc t", p=P)
    wkh_r = wkh.rearrange("(kc p) m -> p kc m", p=P)
    wkl_r = wkl.rearrange("(kc p) m -> p kc m", p=P)
    wvh_r = wvh.rearrange("(kc p) m -> p kc m", p=P)
    wvl_r = wvl.rearrange("(kc p) m -> p kc m", p=P)
    woh_r = woh.rearrange("(h p) c -> p h c", p=P)
    wol_r = wol.rearrange("(h p) c -> p h c", p=P)

    DSQ = 1.0 / (SX * SWQ)      # Q psum descale
    DSKV = 1.0 / (SX * SWKV)    # K/V psum descale
    DSO = 1.0 / (SOT * SWO)     # out-proj psum descale

    with (
        tile.TileContext(nc) as tc,
        tc.tile_pool(name="persist", bufs=1) as persist,
        tc.tile_pool(name="xh", bufs=2) as xhp,
    ):
        QT = persist.tile([P, HPC, seq], bf16)
        KT = persist.tile([P, KVPC, seq], bf16)
        V = persist.tile([P, NKT, KVPC * D], bf16)
        sin_t = persist.tile([P, seq], bf16)
        cos_t = persist.tile([P, seq], bf16)
        cm = persist.tile([P, 4, 512], bf16)
        ones = persist.tile([P, 1], bf16)
        wk8h = persist.tile([P, KC, KVPC * D], fp8)
        wk8l = persist.tile([P, KC, KVPC * D], fp8)
        wv8h = persist.tile([P, KC, KVPC * D], fp8)
        wv8l = persist.tile([P, KC, KVPC * D], fp8)
        wo8h = persist.tile([P, HPC, C], fp8)
        wo8l = persist.tile([P, HPC, C], fp8)
        wk8 = (wk8h, wk8l)
        wv8 = (wv8h, wv8l)
        wo8 = (wo8h, wo8l)

        # l^T is accumulated at 1/SOT so the reciprocal yields SOT/l and
        # the epilogue multiply lands OT pre-scaled for fp8
        nc.vector.memset(ones[:], 1.0 / SOT)

        # ======== phase 1: projections + RoPE ========
        NQ = seq // 512
        DEFER_V = set(range(max(1, NQ - 2), NQ))
        xq_tiles = {}

        def qkv_mms(ps, whi, wlo, xh_, xl_, msl):
            """3-term compensated DoubleRow accumulation into ps.
            whi/wlo: [P, KC, *] weight tiles (stationary, free slice msl);
            xh_/xl_: [P, KC, 512] x tiles (moving)."""
            terms = ((whi, xh_), (whi, xl_), (wlo, xh_))
            for ti, (wa, xa) in enumerate(terms):
                for jp in range(NP):
                    s2 = slice(2 * jp, 2 * jp + 2)
                    nc.tensor.matmul(
                        ps[:], wa[:, s2, msl], xa[:, s2, :],
                        start=(ti == 0 and jp == 0),
                        stop=(ti == 2 and jp == NP - 1),
                        perf_mode=DR)

        with (
            tc.tile_pool(name="wm", bufs=5) as wmp,
            tc.tile_pool(name="qrp", bufs=4) as qrp,
            tc.tile_pool(name="qsp", bufs=3) as qsp,
            tc.tile_pool(name="rtp", bufs=3) as rtp,
            tc.tile_pool(name="psq", bufs=4, space="PSUM") as psqp,
            tc.tile_pool(name="psv", bufs=2, space="PSUM") as psvp,
        ):
            for hf in range(NQ):
                gch = slice(hf * 512, (hf + 1) * 512)
                xqh = xhp.tile([P, KC, 512], fp8, tag="xqh")
                xql = xhp.tile([P, KC, 512], fp8, tag="xql")
                xq_tiles[hf] = (xqh, xql)

                def dma_xq():
                    for kc0 in range(0, KC, 4):
                        nc.sync.dma_start(
                            xqh[:, kc0:kc0 + 4, :],
                            xhi_r[:, kc0:kc0 + 4, gch])
                    for kc0 in range(0, KC, 4):
                        nc.sync.dma_start(
                            xql[:, kc0:kc0 + 4, :],
                            xlo_r[:, kc0:kc0 + 4, gch])

                def dma_wq(m):
                    wah = wmp.tile([P, KC, P], fp8, tag="wmh")
                    nc.sync.dma_start(
                        wah[:], wqh[m].rearrange("p (kc c) -> p kc c", c=P))
                    wal = wmp.tile([P, KC, P], fp8, tag="wml")
                    nc.sync.dma_start(
                        wal[:], wql[m].rearrange("p (kc c) -> p kc c", c=P))
                    return (wah, wal)

                if hf > 0:
                    # out-projection weights: stream on the ACT ring, gated
                    # behind phase-1 ACT work so they stay clear of the
                    # phase-1 SP DMA burst
                    for hh in range((hf - 1) * HPC // NQ, hf * HPC // NQ):
                        nc.gpsimd.dma_start(
                            wo8[0][:, hh:hh + 1, :], woh_r[:, hh:hh + 1, :])
                        nc.gpsimd.dma_start(
                            wo8[1][:, hh:hh + 1, :], wol_r[:, hh:hh + 1, :])
                if hf == 0:
                    # critical-path order: first weights, first x chunks,
                    # then tables and the resident K/V weights
                    wq_pre = {0: dma_wq(0)}
                    for kc0 in range(0, KC, 4):
                        nc.sync.dma_start(
                            xqh[:, kc0:kc0 + 4, :],
                            xhi_r[:, kc0:kc0 + 4, gch])
                        if kc0 == 0:
                            wq_pre[1] = dma_wq(1)
                        if kc0 == 8:
                            wq_pre[2] = dma_wq(2)
                    for kc0 in range(0, KC, 4):
                        nc.sync.dma_start(
                            xql[:, kc0:kc0 + 4, :],
                            xlo_r[:, kc0:kc0 + 4, gch])
                    wq_pre[3] = dma_wq(3)
                    wq_pre[4] = dma_wq(4)
                    nc.sync.dma_start(sin_t[:], sinT[:])
                    nc.sync.dma_start(cos_t[:], cosT[:])
                    # first two head chains: hh term column-major over kc
                    # pair groups so PE tracks the arriving xhi chunks, then
                    # the hl/lh terms once xlo lands
                    ps01 = []
                    for _i in range(2):
                        psq_cm = psqp.tile([P, 512], f32, tag="pq")
                        ps01.append(psq_cm)
                    for jp0 in range(0, NP, 2):
                        for m in range(2):
                            wah, _ = wq_pre[m]
                            for jp in range(jp0, jp0 + 2):
                                s2 = slice(2 * jp, 2 * jp + 2)
                                nc.tensor.matmul(
                                    ps01[m][:], wah[:, s2, :],
                                    xqh[:, s2, :], start=(jp == 0),
                                    stop=False, perf_mode=DR)
                    for m in range(2):
                        wah, wal = wq_pre[m]
                        for ti, wa_xa in enumerate(((wah, xql), (wal, xqh))):
                            wa, xa = wa_xa
                            for jp in range(NP):
                                s2 = slice(2 * jp, 2 * jp + 2)
                                nc.tensor.matmul(
                                    ps01[m][:], wa[:, s2, :], xa[:, s2, :],
                                    start=False,
                                    stop=(ti == 1 and jp == NP - 1),
                                    perf_mode=DR)
                    wq_head = {0: ps01[0], 1: ps01[1]}
                else:
                    dma_xq()
                    wq_pre = {}
                    wq_head = {}

                # Q (streamed weights) then K (resident) projections + RoPE
                for nm, ws, dstT, dsc in ((HPC, None, QT, DSQ),
                                          (KVPC, wk8, KT, DSKV)):
                    for m in range(nm):
                        if ws is None and hf == 0:
                            if m + 2 < HPC and (m + 2) not in wq_pre:
                                wq_pre[m + 2] = dma_wq(m + 2)
                            if m == HPC - 2:
                                nc.sync.dma_start(wk8[0][:], wkh_r[:])
                                nc.sync.dma_start(wk8[1][:], wkl_r[:])
                                nc.sync.dma_start(wv8[0][:], wvh_r[:])
                                nc.sync.dma_start(wv8[1][:], wvl_r[:])
                                nc.sync.dma_start(cm[:], cmask[:])
                        if ws is None and m in wq_head:
                            psq = wq_head[m]
                        else:
                            psq = psqp.tile([P, 512], f32, tag="pq")
                            if ws is None:
                                wah, wal = (wq_pre[m] if m in wq_pre
                                            else dma_wq(m))
                                qkv_mms(psq, wah, wal, xqh, xql,
                                        slice(None))
                            else:
                                qkv_mms(psq, ws[0], ws[1], xqh, xql,
                                        slice(m * P, (m + 1) * P))
                        qr = qrp.tile([P, 512], bf16, tag="qr")
                        nc.scalar.activation(qr[:], psq[:], AF.Copy,
                                             scale=dsc)
                        qs = qsp.tile([P, 512], bf16, tag="qs")
                        nc.gpsimd.dma_start(qs[0:64, :], qr[64:128, :])
                        nc.gpsimd.dma_start(qs[64:128, :], qr[0:64, :])
                        rt = rtp.tile([P, 512], bf16, tag="rt")
                        nc.vector.tensor_tensor(
                            rt[:], qs[:], sin_t[:, gch], ALU.mult)
                        nc.vector.tensor_tensor(
                            dstT[:, m, gch], qr[:], cos_t[:, gch], ALU.mult)
                        nc.vector.tensor_tensor(
                            dstT[:, m, gch], dstT[:, m, gch], rt[:], ALU.add)

                # V projection; the last chunks are deferred into the start
                # of the attention phase to fill its PE bubbles
                if hf not in DEFER_V:
                    for tt in range(4):
                        gtt = hf * 4 + tt
                        psv = psvp.tile([P, KVPC * D], f32, tag="pv")
                        tsl = slice(tt * P, (tt + 1) * P)
                        # stationary = x slice, moving = wv
                        for ti, (xa, wa) in enumerate(
                                ((xqh, wv8[0]), (xql, wv8[0]), (xqh, wv8[1]))):
                            for jp in range(NP):
                                s2 = slice(2 * jp, 2 * jp + 2)
                                nc.tensor.matmul(
                                    psv[:], xa[:, s2, tsl], wa[:, s2, :],
                                    start=(ti == 0 and jp == 0),
                                    stop=(ti == 2 and jp == NP - 1),
                                    perf_mode=DR)
                        nc.scalar.activation(V[:, gtt, :], psv[:], AF.Copy,
                                             scale=DSKV)

        start_h = (NQ - 1) * HPC // NQ if NQ > 1 else 0
        for hh in range(start_h, HPC):
            nc.gpsimd.dma_start(
                wo8[0][:, hh:hh + 1, :], woh_r[:, hh:hh + 1, :])
            nc.gpsimd.dma_start(
                wo8[1][:, hh:hh + 1, :], wol_r[:, hh:hh + 1, :])

        # ======== phases 2+3: attention + interleaved out-projection ====
        # Chunk order: groups 0 and 1 interleaved (g0 alone is too small to
        # keep PE busy), then groups 2, 3. Out-projection chains for group g
        # are injected into later chunks to fill exp-latency PE bubbles.
        with (
            tc.tile_pool(name="otp", bufs=3) as otp,
            tc.tile_pool(name="pch", bufs=7) as pch,
            tc.tile_pool(name="lsp", bufs=2) as lsp,
            tc.tile_pool(name="lbp", bufs=2) as lbp,
            tc.tile_pool(name="sm2", bufs=4) as sm2,
            tc.tile_pool(name="obp", bufs=3) as obp,
            tc.tile_pool(name="ps_s", bufs=3, space="PSUM") as ps_s,
            tc.tile_pool(name="ps_o", bufs=2, space="PSUM") as ps_o,
            tc.tile_pool(name="ps_l", bufs=1, space="PSUM") as ps_l,
            tc.tile_pool(name="ps3", bufs=2, space="PSUM") as ps3p,
        ):
            OTg = {}        # group -> (OT_hi, OT_lo) fp8 tiles [P, HPC, 512]
            pending = []    # deferred PE work (closures)

            def chain_mms(g, tt, co, ps3, tis):
                csl = slice(co * 512, (co + 1) * 512)
                tsl = slice(tt * P, (tt + 1) * P)
                oth, otl = OTg[g]
                terms = ((oth, wo8[0]), (oth, wo8[1]), (otl, wo8[0]))
                for ti in tis:
                    ota, wa = terms[ti]
                    for hp in range(HPC // 2):
                        s2 = slice(2 * hp, 2 * hp + 2)
                        nc.tensor.matmul(
                            ps3[:], ota[:, s2, tsl], wa[:, s2, csl],
                            start=(ti == 0 and hp == 0),
                            stop=(ti == 2 and hp == HPC // 2 - 1),
                            perf_mode=DR)

            def chain_evac(g, tt, co, ps3):
                csl = slice(co * 512, (co + 1) * 512)
                ob = obp.tile([P, 512], f32, tag="ob")
                nc.vector.tensor_scalar_mul(ob[:], ps3[:], DSO)
                nc.sync.dma_start(
                    out[(4 * g + tt) * P:(4 * g + tt + 1) * P, csl], ob[:])

            def emit_chain(g, tt, co):
                """One out-projection chain: 12 DR matmuls + evac + DMA."""
                ps3 = ps3p.tile([P, 512], f32, tag="po")
                chain_mms(g, tt, co, ps3, (0, 1, 2))
                chain_evac(g, tt, co, ps3)

            def emit_vchain(hf, tt):
                """Deferred V projection for one 128-token tile."""
                xqh, xql = xq_tiles[hf]
                gtt = hf * 4 + tt
                tsl = slice(tt * P, (tt + 1) * P)
                psv = ps3p.tile([P, KVPC * D], f32, tag="po")
                for ti, (xa, wa) in enumerate(
                        ((xqh, wv8[0]), (xql, wv8[0]), (xqh, wv8[1]))):
                    for jp in range(NP):
                        s2 = slice(2 * jp, 2 * jp + 2)
                        nc.tensor.matmul(
                            psv[:], xa[:, s2, tsl], wa[:, s2, :],
                            start=(ti == 0 and jp == 0),
                            stop=(ti == 2 and jp == NP - 1),
                            perf_mode=DR)
                nc.scalar.activation(V[:, gtt, :], psv[:], AF.Copy,
                                     scale=DSKV)

            def drain(k):
                for _ in range(min(k, len(pending))):
                    kind, args = pending.pop(0)
                    if kind == "v":
                        emit_vchain(*args)
                    else:
                        emit_chain(*args)

            for dhf in sorted(DEFER_V):
                for tt in range(4):
                    pending.append(("v", (dhf, tt)))

            order = [(g, h) for g in range(NG) for h in range(HPC)]

            class Chunk:
                def __init__(self, g, h):
                    self.g, self.h = g, h
                    self.kv = h // RPH
                    self.ntk = 4 * (g + 1)
                    self.pso = ps_o.tile([P, 512], f32, tag="o")
                    self.psl = ps_l.tile([P, 4], f32, tag="l")

                def q0_of(self, j):
                    return 128 * max(0, j - 4 * self.g)

                def emit_s(self, j):
                    q0 = self.q0_of(j)
                    pss = ps_s.tile([P, 512], f32, tag="s")
                    nc.tensor.matmul(
                        pss[:, q0:], KT[:, self.kv, j * P:(j + 1) * P],
                        QT[:, self.h, self.g * 512 + q0:(self.g + 1) * 512],
                        start=True, stop=True)
                    u = j - 4 * self.g
                    if u >= 0:
                        # mask is nontrivial only on the diagonal 128 cols
                        nc.vector.tensor_tensor(
                            pss[:, q0:q0 + P], pss[:, q0:q0 + P],
                            cm[:, u, q0:q0 + P], ALU.add)
                    return pss

                def emit_lo(self, i, pss):
                    """exp, O-matmul, and transposed-l matmuls for block i."""
                    g, ntk = self.g, self.ntk
                    q0 = self.q0_of(i)
                    u = i - 4 * g
                    pc = pch.tile([P, 512], bf16, tag="p")
                    nc.scalar.activation(pc[:, q0:], pss[:, q0:], AF.Exp)
                    nc.tensor.matmul(
                        self.pso[:, q0:],
                        V[:, i, self.kv * D:(self.kv + 1) * D],
                        pc[:, q0:], start=(i == 0), stop=(i == ntk - 1))
                    # l^T: one tiny matmul per valid 128-col slice; column j
                    # accumulates over blocks i=0..4g+j
                    j0 = max(0, u)
                    for j in range(j0, 4):
                        nc.tensor.matmul(
                            self.psl[:, j:j + 1],
                            pc[:, j * P:(j + 1) * P], ones[:],
                            start=(i == 0), stop=(i == 4 * g + j))

                def epilogue(self):
                    # psl holds l/SOT transposed [q-in-slice, slice]:
                    # reciprocal -> SOT/l while still transposed (4 elems),
                    # DMA to row layout, broadcast, normalize, and split OT
                    # into fp8 hi/lo for the DoubleRow chains
                    lr4 = lsp.tile([P, 4], bf16, tag="lr4")
                    with nc.allow_low_precision(
                            reason="softmax denom reciprocal in bf16"):
                        nc.vector.reciprocal(lr4[:], self.psl[:])
                    lrc = sm2.tile([1, 4, P], bf16, tag="lrc")
                    for j in range(4):
                        nc.gpsimd.dma_start(lrc[0:1, j, :], lr4[:, j:j + 1])
                    lb = lbp.tile([P, 512], bf16, tag="lb")
                    nc.gpsimd.partition_broadcast(
                        lb[:], lrc[:].rearrange("a j p -> a (j p)"))
                    oth, otl = OTg[self.g]
                    obt = sm2.tile([P, 512], bf16, tag="obt")
                    nc.vector.tensor_tensor(
                        obt[:], self.pso[:], lb[:], ALU.mult)
                    with nc.allow_low_precision(
                            reason="fp8 hi/lo split of OT"):
                        nc.vector.tensor_copy(oth[:, self.h, :], obt[:])
                        nc.vector.tensor_tensor(
                            otl[:, self.h, :], obt[:], oth[:, self.h, :],
                            ALU.subtract)

            # flat block stream with cross-chunk S-prefetch (depth PF)
            PF = 2
            for g, h in order:
                if g not in OTg:
                    ot_gh = otp.tile([P, HPC, 512], fp8, tag="oth")
                    ot_gl = otp.tile([P, HPC, 512], fp8, tag="otl")
                    OTg[g] = (ot_gh, ot_gl)

            flat = []        # (chunk key, local block index)
            for g, h in order:
                for j in range(4 * (g + 1)):
                    flat.append(((g, h), j))

            done_heads = {g: 0 for g in range(NG)}
            live = {}
            spos = 0

            def ensure_chunk(key):
                if key not in live:
                    live[key] = Chunk(*key)
                return live[key]

            def emit_s_at(idx):
                key, j = flat[idx]
                ck = ensure_chunk(key)
                return ck, ck.emit_s(j)

            squeue = []
            for _ in range(min(PF, len(flat))):
                squeue.append(emit_s_at(spos))
                spos += 1
            drain(1)
            for idx in range(len(flat)):
                key, j = flat[idx]
                ck, pss = squeue.pop(0)
                ck.emit_lo(j, pss)
                if spos < len(flat):
                    squeue.append(emit_s_at(spos))
                    spos += 1
                if j == ck.ntk - 1:
                    ck.epilogue()
                    del live[key]
                    g = key[0]
                    done_heads[g] += 1
                    if done_heads[g] == HPC:
                        for tt in range(4):
                            for co in range(C // 512):
                                pending.append(("c", (g, tt, co)))
                    drain(1)
                elif j == ck.ntk - 3:
                    # keep chains in reserve so the last chunks' exp tails
                    # still have PE fill work
                    if idx >= len(flat) - 2 * ck.ntk:
                        drain(2)
                    elif len(pending) > 5:
                        drain(1)
            if len(pending) >= 2 and pending[0][0] == "c" \
                    and pending[1][0] == "c":
                a = pending.pop(0)[1]
                b = pending.pop(0)[1]
                psa = ps3p.tile([P, 512], f32, tag="po")
                psb = ps3p.tile([P, 512], f32, tag="po")
                chain_mms(*a, psa, (0, 1))
                chain_mms(*b, psb, (0, 1))
                chain_mms(*a, psa, (2,))
                chain_evac(*a, psa)
                chain_mms(*b, psb, (2,))
                chain_evac(*b, psb)
            drain(len(pending))

    nc.finalize()
    return nc


def _build_generic(seq=T, causal=False):
    import concourse.bacc as bacc
    import concourse.mybir as mybir
    import concourse.tile as tile

    f32 = mybir.dt.float32
    f32r = mybir.dt.float32r
    i32 = mybir.dt.int32
    AF = mybir.ActivationFunctionType
    ALU = mybir.AluOpType

    NKT = seq // P          # Tk tiles of 128
    NG = seq // 512         # Tq groups of 512
    KC = C // P             # 16 contraction chunks
    RPH = HPC // KVPC       # q heads per kv head

    nc = bacc.Bacc(None, target_bir_lowering=False)

    xT = nc.dram_tensor("xT", [C, seq], f32r, kind="ExternalInput")
    wqT = nc.dram_tensor("wqT", [C, HPC * D], f32r, kind="ExternalInput")
    wkT = nc.dram_tensor("wkT", [C, KVPC * D], f32r, kind="ExternalInput")
    wvT = nc.dram_tensor("wvT", [C, KVPC * D], f32r, kind="ExternalInput")
    woT = nc.dram_tensor("woT", [HPC * D, C], f32r, kind="ExternalInput")
    maskT = nc.dram_tensor("maskT", [seq, seq], f32, kind="ExternalInput")
    ifT = nc.dram_tensor("ifT", [D, seq], f32, kind="ExternalInput")
    rmat = nc.dram_tensor("rmat", [D, D], f32r, kind="ExternalInput")
    out = nc.dram_tensor("out", [seq, C], f32, kind="ExternalOutput")

    xT_r = xT.rearrange("(kc p) t -> p kc t", p=P)
    wqT_r = wqT.rearrange("(kc p) m -> p kc m", p=P)
    wkT_r = wkT.rearrange("(kc p) m -> p kc m", p=P)
    wvT_r = wvT.rearrange("(kc p) m -> p kc m", p=P)
    woT_r = woT.rearrange("(h p) c -> p h c", p=P)

    with (
        tile.TileContext(nc) as tc,
        tc.tile_pool(name="persist", bufs=1) as persist,
        tc.tile_pool(name="small", bufs=4) as small,
    ):
        QT = persist.tile([P, HPC, seq], f32r)
        KT = persist.tile([P, KVPC, seq], f32r)
        V = persist.tile([P, NKT, KVPC * D], f32r)
        rm = persist.tile([P, D], f32r)
        nc.sync.dma_start(rm[:], rmat[:])
        ones32 = small.tile([P, 1], f32)
        nc.vector.memset(ones32[:], 1.0)
        ones = persist.tile([P, 1], f32r)
        nc.vector.tensor_copy(ones[:], ones32[:])

        # ======== phase 1: trig tables, projections, RoPE ========
        with tc.tile_pool(name="ph1", bufs=1) as ph1:
            sinT = ph1.tile([P, seq], f32)
            cosT = ph1.tile([P, seq], f32)
            wv_sb = ph1.tile([P, KC, KVPC * D], f32r)

            # sin/cos tables via range-reduced LUT sin:
            # f = (theta/2pi + shift) mod 1;  sin(2*pi*f)
            with tc.tile_pool(name="trig", bufs=1) as trig:
                tf_ = trig.tile([P, seq], f32, tag="tf")
                nc.sync.dma_start(tf_[:], ifT[:])
                inv2pi = float(1.0 / (2.0 * math.pi))
                for dst, shift in ((sinT, 0.0), (cosT, 0.25)):
                    ty = trig.tile([P, seq], f32, tag="ty")
                    nc.vector.tensor_scalar_mul(ty[:], tf_[:], inv2pi)
                    if shift:
                        nc.vector.tensor_scalar_add(ty[:], ty[:], shift)
                    ti_ = trig.tile([P, seq], i32, tag="ti")
                    nc.vector.tensor_copy(ti_[:], ty[:])
                    tfr = trig.tile([P, seq], f32, tag="tfr")
                    nc.vector.tensor_copy(tfr[:], ti_[:])
                    nc.vector.tensor_tensor(ty[:], ty[:], tfr[:], ALU.subtract)
                    nc.scalar.activation(dst[:], ty[:], AF.Sin,
                                         scale=float(2.0 * math.pi))

            with (
                tc.tile_pool(name="xh", bufs=2) as xhp,
                tc.tile_pool(name="wm", bufs=3) as wmp,
                tc.tile_pool(name="praw", bufs=1) as praw,
                tc.tile_pool(name="ps1", bufs=2, space="PSUM") as ps1,
            ):
                NQ = seq // 512
                for hf in range(NQ):
                    gch = slice(hf * 512, (hf + 1) * 512)
                    xq = xhp.tile([P, KC, 512], f32r, tag="xq")
                    nc.sync.dma_start(xq[:], xT_r[:, :, gch])
                    if hf == 0:
                        nc.sync.dma_start(wv_sb[:], wvT_r[:])

                    # Q then K projections + RoPE
                    for nm, wr, dstT in ((HPC, wqT_r, QT), (KVPC, wkT_r, KT)):
                        for m in range(nm):
                            wa = wmp.tile([P, KC // 2, P], f32r, tag="wm")
                            nc.sync.dma_start(
                                wa[:], wr[:, :KC // 2, m * P:(m + 1) * P])
                            wb = wmp.tile([P, KC // 2, P], f32r, tag="wm")
                            nc.sync.dma_start(
                                wb[:], wr[:, KC // 2:, m * P:(m + 1) * P])
                            psq = ps1.tile([P, 512], f32, tag="pq")
                            for kc in range(KC):
                                wt = wa if kc < KC // 2 else wb
                                nc.tensor.matmul(
                                    psq[:], wt[:, kc % (KC // 2), :],
                                    xq[:, kc, :],
                                    start=(kc == 0), stop=(kc == KC - 1))
                            qr = praw.tile([P, 512], f32r, tag="qr")
                            nc.scalar.activation(qr[:], psq[:], AF.Copy)
                            psr = ps1.tile([P, 512], f32, tag="pr")
                            nc.tensor.matmul(psr[:], rm[:], qr[:],
                                             start=True, stop=True)
                            # dst = qr*cos + rot*sin (rot*sin in-place)
                            nc.vector.tensor_tensor(
                                psr[:], psr[:], sinT[:, gch], ALU.mult)
                            nc.vector.tensor_tensor(
                                dstT[:, m, gch], qr[:].bitcast(f32),
                                cosT[:, gch], ALU.mult)
                            nc.vector.tensor_tensor(
                                dstT[:, m, gch],
                                dstT[:, m, gch].bitcast(f32), psr[:],
                                ALU.add)

                    # V projection
                    for tt in range(4):
                        gtt = hf * 4 + tt
                        psv = ps1.tile([P, KVPC * D], f32, tag="pv")
                        for kc in range(KC):
                            nc.tensor.matmul(
                                psv[:], xq[:, kc, tt * P:(tt + 1) * P],
                                wv_sb[:, kc, :],
                                start=(kc == 0), stop=(kc == KC - 1))
                        nc.scalar.activation(V[:, gtt, :], psv[:], AF.Copy)

        # ======== phases 2+3 ========
        with tc.tile_pool(name="otp", bufs=1) as otp:
            OT = otp.tile([P, HPC, seq], f32r)

            with (
                tc.tile_pool(name="mb", bufs=2) as mbp,
                tc.tile_pool(name="pch", bufs=3) as pch,
                tc.tile_pool(name="lbp", bufs=2) as lbp,
                tc.tile_pool(name="sm2", bufs=4) as sm2,
                tc.tile_pool(name="ps_s", bufs=4, space="PSUM") as ps_s,
                tc.tile_pool(name="ps_o", bufs=2, space="PSUM") as ps_o,
                tc.tile_pool(name="ps_l", bufs=2, space="PSUM") as ps_l,
            ):
                for g in range(NG):
                    qsl = slice(g * 512, (g + 1) * 512)
                    nb = 4 if causal else NKT
                    i0 = 4 * g if causal else 0
                    ntk = 4 * (g + 1) if causal else NKT
                    mb = mbp.tile([P, nb, 512], f32, tag="mb")
                    nc.sync.dma_start(
                        mb[:],
                        maskT[i0 * P:(i0 + nb) * P, qsl].rearrange(
                            "(i p) t -> p i t", p=P))
                    for h in range(HPC):
                        kv = h // RPH
                        pso = ps_o.tile([P, 512], f32, tag="o")
                        psl = ps_l.tile([1, 512], f32, tag="l")

                        def emit_s(j):
                            pss = ps_s.tile([P, 512], f32, tag="s")
                            nc.tensor.matmul(
                                pss[:], KT[:, kv, j * P:(j + 1) * P],
                                QT[:, h, qsl], start=True, stop=True)
                            if j >= i0:
                                nc.vector.tensor_tensor(
                                    pss[:], pss[:], mb[:, j - i0, :], ALU.add)
                            return pss

                        sq = [emit_s(j) for j in range(min(2, ntk))]
                        for i in range(ntk):
                            pss = sq.pop(0)
                            pc = pch.tile([P, 512], f32r, tag="p")
                            nc.scalar.activation(pc[:], pss[:], AF.Exp)
                            if i + 2 < ntk:
                                sq.append(emit_s(i + 2))
                            nc.tensor.matmul(
                                psl[:], ones[:], pc[:],
                                start=(i == 0), stop=(i == ntk - 1))
                            nc.tensor.matmul(
                                pso[:], V[:, i, kv * D:(kv + 1) * D], pc[:],
                                start=(i == 0), stop=(i == ntk - 1))
                        lsb = sm2.tile([1, 512], f32, tag="lsb")
                        nc.vector.tensor_copy(lsb[:], psl[:])
                        lrc = sm2.tile([1, 512], f32, tag="lrc")
                        nc.vector.reciprocal(lrc[:], lsb[:])
                        lb = lbp.tile([P, 512], f32, tag="lb")
                        nc.gpsimd.partition_broadcast(lb[:], lrc[:])
                        nc.vector.tensor_tensor(
                            OT[:, h, qsl], pso[:], lb[:], ALU.mult)

            # out-projection, co-chunk outer so weight slices stream once
            with (
                tc.tile_pool(name="wo", bufs=12) as wop,
                tc.tile_pool(name="ob", bufs=3) as obp,
                tc.tile_pool(name="ps3", bufs=4, space="PSUM") as ps3,
            ):
                for co in range(C // 512):
                    csl = slice(co * 512, (co + 1) * 512)
                    woh = []
                    for h in range(HPC):
                        w = wop.tile([P, 512], f32r, tag="wo")
                        nc.sync.dma_start(w[:], woT_r[:, h, csl])
                        woh.append(w)
                    for tt in range(NKT):
                        pso3 = ps3.tile([P, 512], f32, tag="po")
                        for h in range(HPC):
                            nc.tensor.matmul(
                                pso3[:], OT[:, h, tt * P:(tt + 1) * P],
                                woh[h][:],
                                start=(h == 0), stop=(h == HPC - 1))
                        ob = obp.tile([P, 512], f32, tag="ob")
                        nc.scalar.activation(ob[:], pso3[:], AF.Copy)
                        nc.sync.dma_start(
                            out[tt * P:(tt + 1) * P, csl], ob[:])

    nc.finalize()
    return nc


def _prep_in_maps_generic(x, inv_freqs, mask, Wq, Wk, Wv, Wo, seq):
    scale = 1.0 / math.sqrt(D)
    maskT = np.ascontiguousarray(mask.reshape(seq, seq).T)
    ifT = np.ascontiguousarray(inv_freqs.reshape(seq, D).T)
    rmat = _rope_rmat()

    shard = []
    for g in range(2):
        wqT = np.ascontiguousarray((Wq[g * 1024:(g + 1) * 1024, :] * scale).T)
        wkT = np.ascontiguousarray(Wk[g * 256:(g + 1) * 256, :].T)
        wvT = np.ascontiguousarray(Wv[g * 256:(g + 1) * 256, :].T)
        woT = np.ascontiguousarray(Wo[:, g * 1024:(g + 1) * 1024].T)
        shard.append((wqT, wkT, wvT, woT))

    in_maps = []
    for b in range(B):
        xTb = np.ascontiguousarray(x[b].T)
        for g in range(2):
            wqT, wkT, wvT, woT = shard[g]
            in_maps.append({
                "xT": xTb, "wqT": wqT, "wkT": wkT, "wvT": wvT, "woT": woT,
                "maskT": maskT, "ifT": ifT, "rmat": rmat,
            })
    return in_maps


def _get_compiled(seq, causal):
    key = (seq, causal)
    if key not in _compiled:
        if causal:
            _compiled[key] = _build_causal(seq)
        else:
            _compiled[key] = _build_generic(seq)
    return _compiled[key]


def _rope_rmat():
    # lhsT for rot = Pmat @ q, Pmat[2i, 2i+1] = -1, Pmat[2i+1, 2i] = 1:
    # lhsT[d', d] = Pmat[d, d']
    m = np.zeros((D, D), dtype=np.float32)
    for i in range(D // 2):
        m[2 * i + 1, 2 * i] = -1.0
        m[2 * i, 2 * i + 1] = 1.0
    return m


def _canonical_mask():
    # cmask[p, u, t] = 0 if 128u + p <= t else -1e9, for t in [0, 512)
    p = np.arange(P)[:, None, None]
    u = np.arange(4)[None, :, None]
    t = np.arange(512)[None, None, :]
    return np.where(128 * u + p <= t, 0.0, -1e9).astype(np.float32)


def _split8(a, s):
    """hi/lo fp8e4 split of a*s (s a power of 2). Returns (hi, lo)."""
    import ml_dtypes
    f8 = ml_dtypes.float8_e4m3
    a = np.asarray(a, dtype=np.float32) * np.float32(s)
    hi = a.astype(f8)
    lo = (a - hi.astype(np.float32)).astype(f8)
    return hi, lo


def _prep_in_maps_causal(x, inv_freqs, Wq, Wk, Wv, Wo, seq):
    import ml_dtypes
    bf = ml_dtypes.bfloat16
    scale = 1.0 / math.sqrt(D)
    perm = np.concatenate([np.arange(0, D, 2), np.arange(1, D, 2)])
    sign = np.where(np.arange(D) < D // 2, -1.0, 1.0)[:, None]
    ifT = np.ascontiguousarray(inv_freqs.reshape(seq, D).T)
    sinT = np.ascontiguousarray(np.sin(ifT)[perm] * sign).astype(bf)
    cosT = np.ascontiguousarray(np.cos(ifT)[perm]).astype(bf)
    cmask = _canonical_mask().astype(bf)

    shard = []
    for g in range(2):
        wqT = np.ascontiguousarray((Wq[g * 1024:(g + 1) * 1024, :] * scale).T)
        wqT = np.ascontiguousarray(
            wqT.reshape(2048, 8, 128)[:, :, perm].reshape(2048, 1024))
        # tiled layout: [m, p, kc, c] so each head's weights are one
        # contiguous DMA
        wqt = np.ascontiguousarray(
            wqT.reshape(16, 128, 8, 128).transpose(2, 1, 0, 3)
            .reshape(8, 128, 16 * 128))
        wqt_h, wqt_l = _split8(wqt, SWQ)
        wkT = np.ascontiguousarray(Wk[g * 256:(g + 1) * 256, :].T)
        wkT = np.ascontiguousarray(
            wkT.reshape(2048, 2, 128)[:, :, perm].reshape(2048, 256))
        wk_h, wk_l = _split8(wkT, SWKV)
        wvT = np.ascontiguousarray(Wv[g * 256:(g + 1) * 256, :].T)
        wv_h, wv_l = _split8(wvT, SWKV)
        woT = np.ascontiguousarray(Wo[:, g * 1024:(g + 1) * 1024].T)
        wo_h, wo_l = _split8(woT, SWO)
        shard.append((wqt_h, wqt_l, wk_h, wk_l, wv_h, wv_l, wo_h, wo_l))

    in_maps = []
    for b in range(B):
        xTb = np.ascontiguousarray(x[b].T)
        x_h, x_l = _split8(xTb, SX)
        for g in range(2):
            (wqt_h, wqt_l, wk_h, wk_l, wv_h, wv_l, wo_h, wo_l) = shard[g]
            in_maps.append({
                "xhi": x_h, "xlo": x_l,
                "wqh": wqt_h, "wql": wqt_l,
                "wkh": wk_h, "wkl": wk_l,
                "wvh": wv_h, "wvl": wv_l,
                "woh": wo_h, "wol": wo_l,
                "cmask": cmask, "sinT": sinT, "cosT": cosT,
            })
    return in_maps


def _check_causal(mask, seq):
    """True if blocks strictly above the diagonal may be skipped (mask very
    negative -> exp underflows to 0) and blocks at/below the diagonal need
    no mask add (mask exactly 0)."""
    m = mask.reshape(seq, seq)
    iu = np.triu_indices(seq, k=1)
    il = np.tril_indices(seq, k=0)
    return bool((m[iu] <= -1e4).all() and (m[il] == 0.0).all())


def kernel(x, start_pos, inv_freqs, mask, Wq, Wk, Wv, Wo, bo, _trace=False):
    from concourse.bass_utils import run_bass_kernel_spmd

    x = np.asarray(x, dtype=np.float32)
    inv_freqs = np.asarray(inv_freqs, dtype=np.float32)
    mask = np.asarray(mask, dtype=np.float32)
    Wq = np.asarray(Wq, dtype=np.float32)
    Wk = np.asarray(Wk, dtype=np.float32)
    Wv = np.asarray(Wv, dtype=np.float32)
    Wo = np.asarray(Wo, dtype=np.float32)
    bo = np.asarray(bo, dtype=np.float32)

    seq = x.shape[1]
    causal = _check_causal(mask, seq)
    nc = _get_compiled(seq, causal)
    if causal:
        in_maps = _prep_in_maps_causal(x, inv_freqs, Wq, Wk, Wv, Wo, seq)
    else:
        in_maps = _prep_in_maps_generic(x, inv_freqs, mask, Wq, Wk, Wv, Wo,
                                        seq)

    res = run_bass_kernel_spmd(nc, in_maps, core_ids=list(range(8)),
                               trace=_trace)
    outs = [r["out"] for r in res.results]
    y = np.empty((B, seq, C), dtype=np.float32)
    for b in range(B):
        y[b] = outs[2 * b] + outs[2 * b + 1] + bo[None, :]
    if _trace:
        kernel._last_results = res
    return y


# revision 59
# speedup vs baseline: 1.1886x; 1.0335x over previous
"""GQA attention kernel v3 for Trainium2, 8 NeuronCores.

Problem: B=4, T=2048, C=2048, H=16 q-heads, KVH=4 kv-heads, D=128, RoPE,
causal mask, out-projection with bias.

Sharding: (batch b, q-head-group g) -> core 2*b+g. Each core handles one
batch and 8 q heads (= 2 kv heads). The out-projection partial sums of the
two head-groups of a batch are summed on the host (+ bias).

v3 changes vs v2 (409254 -> ~344320 ns cost-model):
  - Q/K/V and out-projections run in fp8e4 (e4m3) DoubleRow perf mode
    (0.5 PE cycles/row, 2x128 contraction per matmul) with a 3-term
    hi/lo error-compensated split: W ~ (Wh + Wl)/sw, x ~ (xh + xl)/sx
    with power-of-2 scales so all terms share one PSUM descale (applied
    in the ACT evacuation). Bit-exact-to-bf16-grade accuracy at 0.75x
    the bf16 PE cost. Scales keep all fp8 values in e4m3's normal range
    (naive splits hit denormals and lose everything).
  - softmax denominator via "transposed-l" matmuls: per 128-col slice of
    each prob block, a [128,128]-stationary x [128,1]-ones matmul gives
    l^T in PSUM at ~1 PE row each (replaces the fat ones-matmuls and the
    whole DVE l-accumulation chain). One PSUM accumulation group per
    chunk: start marks the whole zero region, each column's first write
    lazily zeroes (per-column groups violate zero-region semantics).
  - scores S and attn*V stay bf16 (128-deep contraction does not pair;
    direct fp8 was measured at ~3.6e-2 error, over the 2e-2 budget).
  - Wq resident in SBUF (fp8 halves it); x/xlo split across the SP and
    ACT DMA rings; PE p-state warmup during the first DMA wait; ACT
    evacuation for tail out-projection chains (ACT idles after exp).

Per-core dataflow (feature-major "T" layouts):
  xT  [C, T] fp8 hi/lo  activations (scaled by 16)
  QT  [128, 8, T] bf16  q projections, partition = d (RoPE'd)
  KT  [128, 2, T] bf16  keys
  V   [128, NKT, 256]   values bf16, partition = token within 128-tile
  S^T [Tk, Tq]          scores transposed; exp on ACT; l^T via tiny
                        matmuls; normalization in O^T epilogue
  OT  [128, 8, 512]     fp8 hi/lo (x16), for DoubleRow out-projection
  out [T, C]            partial out-projection (bf16, summed on host)
"""

import math

import numpy as np

B, T, C = 4, 2048, 2048
H, KVH, D = 16, 4, 128
HPC = 8      # q heads per core
KVPC = 2     # kv heads per core
P = 128

SX = 16.0        # x scale
SWQ = 8192.0     # Wq*(1/sqrt(D)) scale
SWKV = 1024.0    # Wk/Wv scale
SWO = 1024.0     # Wo scale
SOT = 16.0       # OT scale (folded into the ones tile: 1/SOT)

_compiled = {}


def _build_causal(seq=T):
    import concourse.bacc as bacc
    import concourse.mybir as mybir
    import concourse.tile as tile

    f32 = mybir.dt.float32
    bf16 = mybir.dt.bfloat16
    fp8 = mybir.dt.float8e4
    AF = mybir.ActivationFunctionType
    ALU = mybir.AluOpType
    DR = mybir.MatmulPerfMode.DoubleRow

    NKT = seq // P          # k tiles of 128
    NG = seq // 512         # q groups of 512
    KC = C // P             # 16 contraction chunks
    NP = KC // 2            # 8 contraction pair-chunks
    RPH = HPC // KVPC       # q heads per kv head

    nc = bacc.Bacc(None, target_bir_lowering=False)

    xhi = nc.dram_tensor("xhi", [C, seq], fp8, kind="ExternalInput")
    xlo = nc.dram_tensor("xlo", [C, seq], fp8, kind="ExternalInput")
    wqh = nc.dram_tensor("wqh", [HPC, P, KC * P], fp8, kind="ExternalInput")
    wql = nc.dram_tensor("wql", [HPC, P, KC * P], fp8, kind="ExternalInput")
    wkh = nc.dram_tensor("wkh", [C, KVPC * D], fp8, kind="ExternalInput")
    wkl = nc.dram_tensor("wkl", [C, KVPC * D], fp8, kind="ExternalInput")
    wvh = nc.dram_tensor("wvh", [C, KVPC * D], fp8, kind="ExternalInput")
    wvl = nc.dram_tensor("wvl", [C, KVPC * D], fp8, kind="ExternalInput")
    woh = nc.dram_tensor("woh", [HPC * D, C], fp8, kind="ExternalInput")
    wol = nc.dram_tensor("wol", [HPC * D, C], fp8, kind="ExternalInput")
    cmask = nc.dram_tensor("cmask", [P, 4, 512], bf16, kind="ExternalInput")
    sinT = nc.dram_tensor("sinT", [D, seq], bf16, kind="ExternalInput")
    cosT = nc.dram_tensor("cosT", [D, seq], bf16, kind="ExternalInput")
    out = nc.dram_tensor("out", [seq, C], f32, kind="ExternalOutput")

    xhi_r = xhi.rearrange("(kc p) t -> p kc t", p=P)
    xlo_r = xlo.rearrange("(kc p) t -> p kc t", p=P)
    wkh_r = wkh.rearrange("(kc p) m -> p kc m", p=P)
    wkl_r = wkl.rearrange("(kc p) m -> p kc m", p=P)
    wvh_r = wvh.rearrange("(kc p) m -> p kc m", p=P)
    wvl_r = wvl.rearrange("(kc p) m -> p kc m", p=P)
    woh_r = woh.rearrange("(h p) c -> p h c", p=P)
    wol_r = wol.rearrange("(h p) c -> p h c", p=P)

    DSQ = 1.0 / (SX * SWQ)      # Q psum descale
    DSKV = 1.0 / (SX * SWKV)    # K/V psum descale
    DSO = 1.0 / (SOT * SWO)     # out-proj psum descale

    with (
        tile.TileContext(nc) as tc,
        tc.tile_pool(name="persist", bufs=1) as persist,
        tc.tile_pool(name="xh", bufs=2) as xhp,
    ):
        QT = persist.tile([P, HPC, seq], bf16)
        KT = persist.tile([P, KVPC, seq], bf16)
        V = persist.tile([P, NKT, KVPC * D], bf16)
        sin_t = persist.tile([P, seq], bf16)
        cos_t = persist.tile([P, seq], bf16)
        cm = persist.tile([P, 4, 512], bf16)
        ones = persist.tile([P, 1], bf16)
        wk8h = persist.tile([P, KC, KVPC * D], fp8)
        wk8l = persist.tile([P, KC, KVPC * D], fp8)
        wv8h = persist.tile([P, KC, KVPC * D], fp8)
        wv8l = persist.tile([P, KC, KVPC * D], fp8)
        wo8h = persist.tile([P, HPC, C], fp8)
        wo8l = persist.tile([P, HPC, C], fp8)
        wk8 = (wk8h, wk8l)
        wv8 = (wv8h, wv8l)
        wo8 = (wo8h, wo8l)

        # l^T is accumulated at 1/SOT so the reciprocal yields SOT/l and
        # the epilogue multiply lands OT pre-scaled for fp8
        nc.vector.memset(ones[:], 1.0 / SOT)

        # ======== phase 1: projections + RoPE ========
        NQ = seq // 512
        DEFER_V = set(range(max(1, NQ - 2), NQ))
        xq_tiles = {}

        def qkv_mms(ps, whi, wlo, xh_, xl_, msl):
            """3-term compensated DoubleRow accumulation into ps.
            whi/wlo: [P, KC, *] weight tiles (stationary, free slice msl);
            xh_/xl_: [P, KC, 512] x tiles (moving)."""
            terms = ((whi, xh_), (whi, xl_), (wlo, xh_))
            for ti, (wa, xa) in enumerate(terms):
                for jp in range(NP):
                    s2 = slice(2 * jp, 2 * jp + 2)
                    nc.tensor.matmul(
                        ps[:], wa[:, s2, msl], xa[:, s2, :],
                        start=(ti == 0 and jp == 0),
                        stop=(ti == 2 and jp == NP - 1),
                        perf_mode=DR)

        with (
            tc.tile_pool(name="wqres", bufs=1) as wqres,
            tc.tile_pool(name="qrp", bufs=4) as qrp,
            tc.tile_pool(name="qsp", bufs=3) as qsp,
            tc.tile_pool(name="rtp", bufs=3) as rtp,
            tc.tile_pool(name="psq", bufs=4, space="PSUM") as psqp,
            tc.tile_pool(name="psv", bufs=2, space="PSUM") as psvp,
        ):
            # resident fp8 Wq tiles (loaded once during chunk 0)
            wq_res = []
            for m in range(HPC):
                wqr_h = wqres.tile([P, KC, P], fp8, tag=f"wqh{m}")
                wqr_l = wqres.tile([P, KC, P], fp8, tag=f"wql{m}")
                wq_res.append((wqr_h, wqr_l))

            # PE p-state warmup: harmless matmuls on a memset tile while the
            # first DMAs are in flight (results never read)
            wu = wqres.tile([P, 512], bf16, tag="warm")
            nc.vector.memset(wu[:], 0.0)
            psw = psqp.tile([P, 512], f32, tag="pq")
            for wi in range(12):
                nc.tensor.matmul(psw[:], wu[:, 0:P], wu[:],
                                 start=(wi == 0), stop=(wi == 11))

            for hf in range(NQ):
                gch = slice(hf * 512, (hf + 1) * 512)
                xqh = xhp.tile([P, KC, 512], fp8, tag="xqh")
                xql = xhp.tile([P, KC, 512], fp8, tag="xql")
                xq_tiles[hf] = (xqh, xql)

                def dma_xq():
                    for kc0 in range(0, KC, 8):
                        nc.sync.dma_start(
                            xqh[:, kc0:kc0 + 8, :],
                            xhi_r[:, kc0:kc0 + 8, gch])
                    for kc0 in range(0, KC, 8):
                        nc.scalar.dma_start(
                            xql[:, kc0:kc0 + 8, :],
                            xlo_r[:, kc0:kc0 + 8, gch])

                def dma_wq(m):
                    wah, wal = wq_res[m]
                    nc.sync.dma_start(
                        wah[:], wqh[m].rearrange("p (kc c) -> p kc c", c=P))
                    nc.sync.dma_start(
                        wal[:], wql[m].rearrange("p (kc c) -> p kc c", c=P))
                    return (wah, wal)

                if hf > 0:
                    # out-projection weights: stream on the ACT ring, gated
                    # behind phase-1 ACT work so they stay clear of the
                    # phase-1 SP DMA burst
                    for hh in range((hf - 1) * HPC // NQ, hf * HPC // NQ):
                        nc.sync.dma_start(
                            wo8[0][:, hh:hh + 1, :], woh_r[:, hh:hh + 1, :])
                        nc.sync.dma_start(
                            wo8[1][:, hh:hh + 1, :], wol_r[:, hh:hh + 1, :])
                if hf == 0:
                    # critical-path order: first weights, x quarters hi then
                    # lo interleaved, then tables and resident K/V weights
                    wq_pre = {0: dma_wq(0)}
                    for kc0, sz in ((0, 4), (4, 4), (8, 8)):
                        nc.sync.dma_start(
                            xqh[:, kc0:kc0 + sz, :],
                            xhi_r[:, kc0:kc0 + sz, gch])
                        nc.scalar.dma_start(
                            xql[:, kc0:kc0 + sz, :],
                            xlo_r[:, kc0:kc0 + sz, gch])
                        if kc0 == 0:
                            wq_pre[1] = dma_wq(1)
                    wq_pre[2] = dma_wq(2)
                    wq_pre[3] = dma_wq(3)
                    wq_pre[4] = dma_wq(4)
                    nc.scalar.dma_start(sin_t[:], sinT[:])
                    nc.scalar.dma_start(cos_t[:], cosT[:])
                    # first two head chains: all 3 terms column-major over
                    # kc pair quarters so PE tracks the arriving x chunks
                    ps01 = []
                    for _i in range(2):
                        psq_cm = psqp.tile([P, 512], f32, tag="pq")
                        ps01.append(psq_cm)
                    for jp0 in range(0, NP, 2):
                        for m in range(2):
                            wah, wal = wq_pre[m]
                            for ti, (wa, xa) in enumerate(
                                    ((wah, xqh), (wah, xql), (wal, xqh))):
                                for jp in range(jp0, jp0 + 2):
                                    s2 = slice(2 * jp, 2 * jp + 2)
                                    nc.tensor.matmul(
                                        ps01[m][:], wa[:, s2, :],
                                        xa[:, s2, :],
                                        start=(ti == 0 and jp == 0),
                                        stop=(ti == 2 and jp == NP - 1),
                                        perf_mode=DR)
                    wq_head = {0: ps01[0], 1: ps01[1]}
                else:
                    dma_xq()
                    wq_pre = {m: wq_res[m] for m in range(HPC)}
                    wq_head = {}

                # Q (streamed weights) then K (resident) projections + RoPE
                for nm, ws, dstT, dsc in ((HPC, None, QT, DSQ),
                                          (KVPC, wk8, KT, DSKV)):
                    for m in range(nm):
                        if ws is None and hf == 0:
                            if m + 2 < HPC and (m + 2) not in wq_pre:
                                wq_pre[m + 2] = dma_wq(m + 2)
                            if m == HPC - 2:
                                nc.scalar.dma_start(wk8[0][:], wkh_r[:])
                                nc.scalar.dma_start(wk8[1][:], wkl_r[:])
                                nc.scalar.dma_start(wv8[0][:], wvh_r[:])
                                nc.scalar.dma_start(wv8[1][:], wvl_r[:])
                                nc.scalar.dma_start(cm[:], cmask[:])
                        if ws is None and m in wq_head:
                            psq = wq_head[m]
                        else:
                            psq = psqp.tile([P, 512], f32, tag="pq")
                            if ws is None:
                                wah, wal = (wq_pre[m] if m in wq_pre
                                            else dma_wq(m))
                                qkv_mms(psq, wah, wal, xqh, xql,
                                        slice(None))
                            else:
                                qkv_mms(psq, ws[0], ws[1], xqh, xql,
                                        slice(m * P, (m + 1) * P))
                        qr = qrp.tile([P, 512], bf16, tag="qr")
                        nc.scalar.activation(qr[:], psq[:], AF.Copy,
                                             scale=dsc)
                        qs = qsp.tile([P, 512], bf16, tag="qs")
                        nc.gpsimd.dma_start(qs[0:64, :], qr[64:128, :])
                        nc.gpsimd.dma_start(qs[64:128, :], qr[0:64, :])
                        rt = rtp.tile([P, 512], bf16, tag="rt")
                        nc.vector.tensor_tensor(
                            rt[:], qs[:], sin_t[:, gch], ALU.mult)
                        nc.vector.tensor_tensor(
                            dstT[:, m, gch], qr[:], cos_t[:, gch], ALU.mult)
                        nc.vector.tensor_tensor(
                            dstT[:, m, gch], dstT[:, m, gch], rt[:], ALU.add)

                # V projection; the last chunks are deferred into the start
                # of the attention phase to fill its PE bubbles
                if hf not in DEFER_V:
                    for tt in range(4):
                        gtt = hf * 4 + tt
                        psv = psvp.tile([P, KVPC * D], f32, tag="pv")
                        tsl = slice(tt * P, (tt + 1) * P)
                        # stationary = x slice, moving = wv
                        for ti, (xa, wa) in enumerate(
                                ((xqh, wv8[0]), (xql, wv8[0]), (xqh, wv8[1]))):
                            for jp in range(NP):
                                s2 = slice(2 * jp, 2 * jp + 2)
                                nc.tensor.matmul(
                                    psv[:], xa[:, s2, tsl], wa[:, s2, :],
                                    start=(ti == 0 and jp == 0),
                                    stop=(ti == 2 and jp == NP - 1),
                                    perf_mode=DR)
                        nc.scalar.activation(V[:, gtt, :], psv[:], AF.Copy,
                                             scale=DSKV)

        start_h = (NQ - 1) * HPC // NQ if NQ > 1 else 0
        for hh in range(start_h, HPC):
            nc.sync.dma_start(
                wo8[0][:, hh:hh + 1, :], woh_r[:, hh:hh + 1, :])
            nc.sync.dma_start(
                wo8[1][:, hh:hh + 1, :], wol_r[:, hh:hh + 1, :])

        # ======== phases 2+3: attention + interleaved out-projection ====
        # Chunk order: groups 0 and 1 interleaved (g0 alone is too small to
        # keep PE busy), then groups 2, 3. Out-projection chains for group g
        # are injected into later chunks to fill exp-latency PE bubbles.
        with (
            tc.tile_pool(name="otp", bufs=3) as otp,
            tc.tile_pool(name="pch", bufs=7) as pch,
            tc.tile_pool(name="lsp", bufs=2) as lsp,
            tc.tile_pool(name="lbp", bufs=2) as lbp,
            tc.tile_pool(name="sm2", bufs=4) as sm2,
            tc.tile_pool(name="obp", bufs=3) as obp,
            tc.tile_pool(name="ps_s", bufs=3, space="PSUM") as ps_s,
            tc.tile_pool(name="ps_o", bufs=2, space="PSUM") as ps_o,
            tc.tile_pool(name="ps_l", bufs=1, space="PSUM") as ps_l,
            tc.tile_pool(name="ps3", bufs=2, space="PSUM") as ps3p,
        ):
            OTg = {}        # group -> (OT_hi, OT_lo) fp8 tiles [P, HPC, 512]
            pending = []    # deferred PE work (closures)

            def chain_mms(g, tt, co, ps3, tis):
                csl = slice(co * 512, (co + 1) * 512)
                tsl = slice(tt * P, (tt + 1) * P)
                oth, otl = OTg[g]
                terms = ((oth, wo8[0]), (oth, wo8[1]), (otl, wo8[0]))
                for ti in tis:
                    ota, wa = terms[ti]
                    for hp in range(HPC // 2):
                        s2 = slice(2 * hp, 2 * hp + 2)
                        nc.tensor.matmul(
                            ps3[:], ota[:, s2, tsl], wa[:, s2, csl],
                            start=(ti == 0 and hp == 0),
                            stop=(ti == 2 and hp == HPC // 2 - 1),
                            perf_mode=DR)

            def chain_evac(g, tt, co, ps3, act=False):
                csl = slice(co * 512, (co + 1) * 512)
                ob = obp.tile([P, 512], bf16, tag="ob")
                if act:
                    nc.scalar.activation(ob[:], ps3[:], AF.Copy, scale=DSO)
                else:
                    nc.vector.tensor_scalar_mul(ob[:], ps3[:], DSO)
                nc.sync.dma_start(
                    out[(4 * g + tt) * P:(4 * g + tt + 1) * P, csl], ob[:])

            def emit_chain(g, tt, co, act=False):
                """One out-projection chain: 12 DR matmuls + evac + DMA."""
                ps3 = ps3p.tile([P, 512], f32, tag="po")
                chain_mms(g, tt, co, ps3, (0, 1, 2))
                chain_evac(g, tt, co, ps3, act=act)

            def emit_vchain(hf, tt):
                """Deferred V projection for one 128-token tile."""
                xqh, xql = xq_tiles[hf]
                gtt = hf * 4 + tt
                tsl = slice(tt * P, (tt + 1) * P)
                psv = ps3p.tile([P, KVPC * D], f32, tag="po")
                for ti, (xa, wa) in enumerate(
                        ((xqh, wv8[0]), (xql, wv8[0]), (xqh, wv8[1]))):
                    for jp in range(NP):
                        s2 = slice(2 * jp, 2 * jp + 2)
                        nc.tensor.matmul(
                            psv[:], xa[:, s2, tsl], wa[:, s2, :],
                            start=(ti == 0 and jp == 0),
                            stop=(ti == 2 and jp == NP - 1),
                            perf_mode=DR)
                nc.scalar.activation(V[:, gtt, :], psv[:], AF.Copy,
                                     scale=DSKV)

            def drain(k, act=False):
                for _ in range(min(k, len(pending))):
                    kind, args = pending.pop(0)
                    if kind == "v":
                        emit_vchain(*args)
                    else:
                        emit_chain(*args, act=act)

            for dhf in sorted(DEFER_V):
                for tt in range(4):
                    pending.append(("v", (dhf, tt)))

            order = [(g, h) for g in range(NG) for h in range(HPC)]

            # single [P, 4] PSUM tile for l^T; one accumulation group per
            # chunk (PSUM zero-region semantics forbid per-column groups)
            psl4 = ps_l.tile([P, 4], f32)

            class Chunk:
                def __init__(self, g, h):
                    self.g, self.h = g, h
                    self.kv = h // RPH
                    self.ntk = 4 * (g + 1)
                    self.pso = ps_o.tile([P, 512], f32, tag="o")

                def q0_of(self, j):
                    return 128 * max(0, j - 4 * self.g)

                def emit_s(self, j):
                    q0 = self.q0_of(j)
                    pss = ps_s.tile([P, 512], f32, tag="s")
                    nc.tensor.matmul(
                        pss[:, q0:], KT[:, self.kv, j * P:(j + 1) * P],
                        QT[:, self.h, self.g * 512 + q0:(self.g + 1) * 512],
                        start=True, stop=True)
                    u = j - 4 * self.g
                    if u >= 0:
                        # mask is nontrivial only on the diagonal 128 cols
                        nc.vector.tensor_tensor(
                            pss[:, q0:q0 + P], pss[:, q0:q0 + P],
                            cm[:, u, q0:q0 + P], ALU.add)
                    return pss

                def emit_lo(self, i, pss):
                    """exp, O-matmul, and transposed-l matmuls for block i."""
                    g, ntk = self.g, self.ntk
                    q0 = self.q0_of(i)
                    u = i - 4 * g
                    pc = pch.tile([P, 512], bf16, tag="p")
                    nc.scalar.activation(pc[:, q0:], pss[:, q0:], AF.Exp)
                    nc.tensor.matmul(
                        self.pso[:, q0:],
                        V[:, i, self.kv * D:(self.kv + 1) * D],
                        pc[:, q0:], start=(i == 0), stop=(i == ntk - 1))
                    # l^T: one tiny matmul per valid 128-col slice; a single
                    # psum group per chunk (start marks the zero region, each
                    # column's first write lazily zeroes its bytes)
                    j0 = max(0, u)
                    for j in range(j0, 4):
                        nc.tensor.matmul(
                            psl4[:, j:j + 1],
                            pc[:, j * P:(j + 1) * P], ones[:],
                            start=(i == 0 and j == 0),
                            stop=(i == ntk - 1 and j == 3))

                def epilogue(self):
                    # psl holds l/SOT transposed [q-in-slice, slice]:
                    # reciprocal -> SOT/l while still transposed (4 elems),
                    # DMA to row layout, broadcast, normalize, and split OT
                    # into fp8 hi/lo for the DoubleRow chains
                    lr4 = lsp.tile([P, 4], bf16, tag="lr4")
                    with nc.allow_low_precision(
                            reason="softmax denom reciprocal in bf16"):
                        nc.vector.reciprocal(lr4[:], psl4[:])
                    lrc = sm2.tile([1, 4, P], bf16, tag="lrc")
                    for j in range(4):
                        nc.gpsimd.dma_start(lrc[0:1, j, :], lr4[:, j:j + 1])
                    lb = lbp.tile([P, 512], bf16, tag="lb")
                    nc.gpsimd.partition_broadcast(
                        lb[:], lrc[:].rearrange("a j p -> a (j p)"))
                    oth, otl = OTg[self.g]
                    obt = sm2.tile([P, 512], bf16, tag="obt")
                    nc.vector.tensor_tensor(
                        obt[:], self.pso[:], lb[:], ALU.mult)
                    with nc.allow_low_precision(
                            reason="fp8 hi/lo split of OT"):
                        nc.vector.tensor_copy(oth[:, self.h, :], obt[:])
                        nc.vector.tensor_tensor(
                            otl[:, self.h, :], obt[:], oth[:, self.h, :],
                            ALU.subtract)

            # flat block stream with cross-chunk S-prefetch (depth PF)
            PF = 2
            for g, h in order:
                if g not in OTg:
                    ot_gh = otp.tile([P, HPC, 512], fp8, tag="oth")
                    ot_gl = otp.tile([P, HPC, 512], fp8, tag="otl")
                    OTg[g] = (ot_gh, ot_gl)

            flat = []        # (chunk key, local block index)
            for g, h in order:
                for j in range(4 * (g + 1)):
                    flat.append(((g, h), j))

            done_heads = {g: 0 for g in range(NG)}
            live = {}
            spos = 0

            def ensure_chunk(key):
                if key not in live:
                    live[key] = Chunk(*key)
                return live[key]

            def emit_s_at(idx):
                key, j = flat[idx]
                ck = ensure_chunk(key)
                return ck, ck.emit_s(j)

            squeue = []
            for _ in range(min(PF, len(flat))):
                squeue.append(emit_s_at(spos))
                spos += 1
            drain(1)
            for idx in range(len(flat)):
                key, j = flat[idx]
                ck, pss = squeue.pop(0)
                ck.emit_lo(j, pss)
                if spos < len(flat):
                    squeue.append(emit_s_at(spos))
                    spos += 1
                if j == ck.ntk - 1:
                    ck.epilogue()
                    del live[key]
                    g = key[0]
                    done_heads[g] += 1
                    if done_heads[g] == HPC:
                        for tt in range(4):
                            for co in range(C // 512):
                                pending.append(("c", (g, tt, co)))
                    if idx < len(flat) - 40 or len(pending) > 4:
                        drain(1)
                elif j == ck.ntk - 3:
                    # keep chains in reserve so the last chunks' exp tails
                    # and the final epilogue latency still have PE fill work
                    if idx < len(flat) - 40 and len(pending) > 5:
                        drain(1)
            if len(pending) >= 2 and pending[0][0] == "c" \
                    and pending[1][0] == "c":
                a = pending.pop(0)[1]
                b = pending.pop(0)[1]
                psa = ps3p.tile([P, 512], f32, tag="po")
                psb = ps3p.tile([P, 512], f32, tag="po")
                chain_mms(*a, psa, (0, 1))
                chain_mms(*b, psb, (0, 1))
                chain_mms(*a, psa, (2,))
                chain_evac(*a, psa)
                chain_mms(*b, psb, (2,))
                chain_evac(*b, psb)
            drain(len(pending), act=True)

    nc.finalize()
    return nc


def _build_generic(seq=T, causal=False):
    import concourse.bacc as bacc
    import concourse.mybir as mybir
    import concourse.tile as tile

    f32 = mybir.dt.float32
    f32r = mybir.dt.float32r
    i32 = mybir.dt.int32
    AF = mybir.ActivationFunctionType
    ALU = mybir.AluOpType

    NKT = seq // P          # Tk tiles of 128
    NG = seq // 512         # Tq groups of 512
    KC = C // P             # 16 contraction chunks
    RPH = HPC // KVPC       # q heads per kv head

    nc = bacc.Bacc(None, target_bir_lowering=False)

    xT = nc.dram_tensor("xT", [C, seq], f32r, kind="ExternalInput")
    wqT = nc.dram_tensor("wqT", [C, HPC * D], f32r, kind="ExternalInput")
    wkT = nc.dram_tensor("wkT", [C, KVPC * D], f32r, kind="ExternalInput")
    wvT = nc.dram_tensor("wvT", [C, KVPC * D], f32r, kind="ExternalInput")
    woT = nc.dram_tensor("woT", [HPC * D, C], f32r, kind="ExternalInput")
    maskT = nc.dram_tensor("maskT", [seq, seq], f32, kind="ExternalInput")
    ifT = nc.dram_tensor("ifT", [D, seq], f32, kind="ExternalInput")
    rmat = nc.dram_tensor("rmat", [D, D], f32r, kind="ExternalInput")
    out = nc.dram_tensor("out", [seq, C], f32, kind="ExternalOutput")

    xT_r = xT.rearrange("(kc p) t -> p kc t", p=P)
    wqT_r = wqT.rearrange("(kc p) m -> p kc m", p=P)
    wkT_r = wkT.rearrange("(kc p) m -> p kc m", p=P)
    wvT_r = wvT.rearrange("(kc p) m -> p kc m", p=P)
    woT_r = woT.rearrange("(h p) c -> p h c", p=P)

    with (
        tile.TileContext(nc) as tc,
        tc.tile_pool(name="persist", bufs=1) as persist,
        tc.tile_pool(name="small", bufs=4) as small,
    ):
        QT = persist.tile([P, HPC, seq], f32r)
        KT = persist.tile([P, KVPC, seq], f32r)
        V = persist.tile([P, NKT, KVPC * D], f32r)
        rm = persist.tile([P, D], f32r)
        nc.sync.dma_start(rm[:], rmat[:])
        ones32 = small.tile([P, 1], f32)
        nc.vector.memset(ones32[:], 1.0)
        ones = persist.tile([P, 1], f32r)
        nc.vector.tensor_copy(ones[:], ones32[:])

        # ======== phase 1: trig tables, projections, RoPE ========
        with tc.tile_pool(name="ph1", bufs=1) as ph1:
            sinT = ph1.tile([P, seq], f32)
            cosT = ph1.tile([P, seq], f32)
            wv_sb = ph1.tile([P, KC, KVPC * D], f32r)

            # sin/cos tables via range-reduced LUT sin:
            # f = (theta/2pi + shift) mod 1;  sin(2*pi*f)
            with tc.tile_pool(name="trig", bufs=1) as trig:
                tf_ = trig.tile([P, seq], f32, tag="tf")
                nc.sync.dma_start(tf_[:], ifT[:])
                inv2pi = float(1.0 / (2.0 * math.pi))
                for dst, shift in ((sinT, 0.0), (cosT, 0.25)):
                    ty = trig.tile([P, seq], f32, tag="ty")
                    nc.vector.tensor_scalar_mul(ty[:], tf_[:], inv2pi)
                    if shift:
                        nc.vector.tensor_scalar_add(ty[:], ty[:], shift)
                    ti_ = trig.tile([P, seq], i32, tag="ti")
                    nc.vector.tensor_copy(ti_[:], ty[:])
                    tfr = trig.tile([P, seq], f32, tag="tfr")
                    nc.vector.tensor_copy(tfr[:], ti_[:])
                    nc.vector.tensor_tensor(ty[:], ty[:], tfr[:], ALU.subtract)
                    nc.scalar.activation(dst[:], ty[:], AF.Sin,
                                         scale=float(2.0 * math.pi))

            with (
                tc.tile_pool(name="xh", bufs=2) as xhp,
                tc.tile_pool(name="wm", bufs=3) as wmp,
                tc.tile_pool(name="praw", bufs=1) as praw,
                tc.tile_pool(name="ps1", bufs=2, space="PSUM") as ps1,
            ):
                NQ = seq // 512
                for hf in range(NQ):
                    gch = slice(hf * 512, (hf + 1) * 512)
                    xq = xhp.tile([P, KC, 512], f32r, tag="xq")
                    nc.sync.dma_start(xq[:], xT_r[:, :, gch])
                    if hf == 0:
                        nc.sync.dma_start(wv_sb[:], wvT_r[:])

                    # Q then K projections + RoPE
                    for nm, wr, dstT in ((HPC, wqT_r, QT), (KVPC, wkT_r, KT)):
                        for m in range(nm):
                            wa = wmp.tile([P, KC // 2, P], f32r, tag="wm")
                            nc.sync.dma_start(
                                wa[:], wr[:, :KC // 2, m * P:(m + 1) * P])
                            wb = wmp.tile([P, KC // 2, P], f32r, tag="wm")
                            nc.sync.dma_start(
                                wb[:], wr[:, KC // 2:, m * P:(m + 1) * P])
                            psq = ps1.tile([P, 512], f32, tag="pq")
                            for kc in range(KC):
                                wt = wa if kc < KC // 2 else wb
                                nc.tensor.matmul(
                                    psq[:], wt[:, kc % (KC // 2), :],
                                    xq[:, kc, :],
                                    start=(kc == 0), stop=(kc == KC - 1))
                            qr = praw.tile([P, 512], f32r, tag="qr")
                            nc.scalar.activation(qr[:], psq[:], AF.Copy)
                            psr = ps1.tile([P, 512], f32, tag="pr")
                            nc.tensor.matmul(psr[:], rm[:], qr[:],
                                             start=True, stop=True)
                            # dst = qr*cos + rot*sin (rot*sin in-place)
                            nc.vector.tensor_tensor(
                                psr[:], psr[:], sinT[:, gch], ALU.mult)
                            nc.vector.tensor_tensor(
                                dstT[:, m, gch], qr[:].bitcast(f32),
                                cosT[:, gch], ALU.mult)
                            nc.vector.tensor_tensor(
                                dstT[:, m, gch],
                                dstT[:, m, gch].bitcast(f32), psr[:],
                                ALU.add)

                    # V projection
                    for tt in range(4):
                        gtt = hf * 4 + tt
                        psv = ps1.tile([P, KVPC * D], f32, tag="pv")
                        for kc in range(KC):
                            nc.tensor.matmul(
                                psv[:], xq[:, kc, tt * P:(tt + 1) * P],
                                wv_sb[:, kc, :],
                                start=(kc == 0), stop=(kc == KC - 1))
                        nc.scalar.activation(V[:, gtt, :], psv[:], AF.Copy)

        # ======== phases 2+3 ========
        with tc.tile_pool(name="otp", bufs=1) as otp:
            OT = otp.tile([P, HPC, seq], f32r)

            with (
                tc.tile_pool(name="mb", bufs=2) as mbp,
                tc.tile_pool(name="pch", bufs=3) as pch,
                tc.tile_pool(name="lbp", bufs=2) as lbp,
                tc.tile_pool(name="sm2", bufs=4) as sm2,
                tc.tile_pool(name="ps_s", bufs=3, space="PSUM") as ps_s,
                tc.tile_pool(name="ps_o", bufs=2, space="PSUM") as ps_o,
                tc.tile_pool(name="ps_l", bufs=2, space="PSUM") as ps_l,
            ):
                for g in range(NG):
                    qsl = slice(g * 512, (g + 1) * 512)
                    nb = 4 if causal else NKT
                    i0 = 4 * g if causal else 0
                    ntk = 4 * (g + 1) if causal else NKT
                    mb = mbp.tile([P, nb, 512], f32, tag="mb")
                    nc.sync.dma_start(
                        mb[:],
                        maskT[i0 * P:(i0 + nb) * P, qsl].rearrange(
                            "(i p) t -> p i t", p=P))
                    for h in range(HPC):
                        kv = h // RPH
                        pso = ps_o.tile([P, 512], f32, tag="o")
                        psl = ps_l.tile([1, 512], f32, tag="l")

                        def emit_s(j):
                            pss = ps_s.tile([P, 512], f32, tag="s")
                            nc.tensor.matmul(
                                pss[:], KT[:, kv, j * P:(j + 1) * P],
                                QT[:, h, qsl], start=True, stop=True)
                            if j >= i0:
                                nc.vector.tensor_tensor(
                                    pss[:], pss[:], mb[:, j - i0, :], ALU.add)
                            return pss

                        sq = [emit_s(j) for j in range(min(2, ntk))]
                        for i in range(ntk):
                            pss = sq.pop(0)
                            pc = pch.tile([P, 512], f32r, tag="p")
                            nc.scalar.activation(pc[:], pss[:], AF.Exp)
                            if i + 2 < ntk:
                                sq.append(emit_s(i + 2))
                            nc.tensor.matmul(
                                psl[:], ones[:], pc[:],
                                start=(i == 0), stop=(i == ntk - 1))
                            nc.tensor.matmul(
                                pso[:], V[:, i, kv * D:(kv + 1) * D], pc[:],
                                start=(i == 0), stop=(i == ntk - 1))
                        lsb = sm2.tile([1, 512], f32, tag="lsb")
                        nc.vector.tensor_copy(lsb[:], psl[:])
                        lrc = sm2.tile([1, 512], f32, tag="lrc")
                        nc.vector.reciprocal(lrc[:], lsb[:])
                        lb = lbp.tile([P, 512], f32, tag="lb")
                        nc.gpsimd.partition_broadcast(lb[:], lrc[:])
                        nc.vector.tensor_tensor(
                            OT[:, h, qsl], pso[:], lb[:], ALU.mult)

            # out-projection, co-chunk outer so weight slices stream once
            with (
                tc.tile_pool(name="wo", bufs=12) as wop,
                tc.tile_pool(name="ob", bufs=3) as obp,
                tc.tile_pool(name="ps3", bufs=4, space="PSUM") as ps3,
            ):
                for co in range(C // 512):
                    csl = slice(co * 512, (co + 1) * 512)
                    woh = []
                    for h in range(HPC):
                        w = wop.tile([P, 512], f32r, tag="wo")
                        nc.sync.dma_start(w[:], woT_r[:, h, csl])
                        woh.append(w)
                    for tt in range(NKT):
                        pso3 = ps3.tile([P, 512], f32, tag="po")
                        for h in range(HPC):
                            nc.tensor.matmul(
                                pso3[:], OT[:, h, tt * P:(tt + 1) * P],
                                woh[h][:],
                                start=(h == 0), stop=(h == HPC - 1))
                        ob = obp.tile([P, 512], f32, tag="ob")
                        nc.scalar.activation(ob[:], pso3[:], AF.Copy)
                        nc.sync.dma_start(
                            out[tt * P:(tt + 1) * P, csl], ob[:])

    nc.finalize()
    return nc


def _prep_in_maps_generic(x, inv_freqs, mask, Wq, Wk, Wv, Wo, seq):
    scale = 1.0 / math.sqrt(D)
    maskT = np.ascontiguousarray(mask.reshape(seq, seq).T)
    ifT = np.ascontiguousarray(inv_freqs.reshape(seq, D).T)
    rmat = _rope_rmat()

    shard = []
    for g in range(2):
        wqT = np.ascontiguousarray((Wq[g * 1024:(g + 1) * 1024, :] * scale).T)
        wkT = np.ascontiguousarray(Wk[g * 256:(g + 1) * 256, :].T)
        wvT = np.ascontiguousarray(Wv[g * 256:(g + 1) * 256, :].T)
        woT = np.ascontiguousarray(Wo[:, g * 1024:(g + 1) * 1024].T)
        shard.append((wqT, wkT, wvT, woT))

    in_maps = []
    for b in range(B):
        xTb = np.ascontiguousarray(x[b].T)
        for g in range(2):
            wqT, wkT, wvT, woT = shard[g]
            in_maps.append({
                "xT": xTb, "wqT": wqT, "wkT": wkT, "wvT": wvT, "woT": woT,
                "maskT": maskT, "ifT": ifT, "rmat": rmat,
            })
    return in_maps


def _get_compiled(seq, causal):
    key = (seq, causal)
    if key not in _compiled:
        if causal:
            _compiled[key] = _build_causal(seq)
        else:
            _compiled[key] = _build_generic(seq)
    return _compiled[key]


def _rope_rmat():
    # lhsT for rot = Pmat @ q, Pmat[2i, 2i+1] = -1, Pmat[2i+1, 2i] = 1:
    # lhsT[d', d] = Pmat[d, d']
    m = np.zeros((D, D), dtype=np.float32)
    for i in range(D // 2):
        m[2 * i + 1, 2 * i] = -1.0
        m[2 * i, 2 * i + 1] = 1.0
    return m


def _canonical_mask():
    # cmask[p, u, t] = 0 if 128u + p <= t else -1e9, for t in [0, 512)
    p = np.arange(P)[:, None, None]
    u = np.arange(4)[None, :, None]
    t = np.arange(512)[None, None, :]
    return np.where(128 * u + p <= t, 0.0, -1e9).astype(np.float32)


def _split8(a, s):
    """hi/lo fp8e4 split of a*s (s a power of 2). Returns (hi, lo)."""
    import ml_dtypes
    f8 = ml_dtypes.float8_e4m3
    a = np.asarray(a, dtype=np.float32) * np.float32(s)
    hi = a.astype(f8)
    lo = (a - hi.astype(np.float32)).astype(f8)
    return hi, lo


def _prep_in_maps_causal(x, inv_freqs, Wq, Wk, Wv, Wo, seq):
    import ml_dtypes
    bf = ml_dtypes.bfloat16
    scale = 1.0 / math.sqrt(D)
    perm = np.concatenate([np.arange(0, D, 2), np.arange(1, D, 2)])
    sign = np.where(np.arange(D) < D // 2, -1.0, 1.0)[:, None]
    ifT = np.ascontiguousarray(inv_freqs.reshape(seq, D).T)
    sinT = np.ascontiguousarray(np.sin(ifT)[perm] * sign).astype(bf)
    cosT = np.ascontiguousarray(np.cos(ifT)[perm]).astype(bf)
    cmask = _canonical_mask().astype(bf)

    shard = []
    for g in range(2):
        wqT = np.ascontiguousarray((Wq[g * 1024:(g + 1) * 1024, :] * scale).T)
        wqT = np.ascontiguousarray(
            wqT.reshape(2048, 8, 128)[:, :, perm].reshape(2048, 1024))
        # tiled layout: [m, p, kc, c] so each head's weights are one
        # contiguous DMA
        wqt = np.ascontiguousarray(
            wqT.reshape(16, 128, 8, 128).transpose(2, 1, 0, 3)
            .reshape(8, 128, 16 * 128))
        wqt_h, wqt_l = _split8(wqt, SWQ)
        wkT = np.ascontiguousarray(Wk[g * 256:(g + 1) * 256, :].T)
        wkT = np.ascontiguousarray(
            wkT.reshape(2048, 2, 128)[:, :, perm].reshape(2048, 256))
        wk_h, wk_l = _split8(wkT, SWKV)
        wvT = np.ascontiguousarray(Wv[g * 256:(g + 1) * 256, :].T)
        wv_h, wv_l = _split8(wvT, SWKV)
        woT = np.ascontiguousarray(Wo[:, g * 1024:(g + 1) * 1024].T)
        wo_h, wo_l = _split8(woT, SWO)
        shard.append((wqt_h, wqt_l, wk_h, wk_l, wv_h, wv_l, wo_h, wo_l))

    in_maps = []
    for b in range(B):
        xTb = np.ascontiguousarray(x[b].T)
        x_h, x_l = _split8(xTb, SX)
        for g in range(2):
            (wqt_h, wqt_l, wk_h, wk_l, wv_h, wv_l, wo_h, wo_l) = shard[g]
            in_maps.append({
                "xhi": x_h, "xlo": x_l,
                "wqh": wqt_h, "wql": wqt_l,
                "wkh": wk_h, "wkl": wk_l,
                "wvh": wv_h, "wvl": wv_l,
                "woh": wo_h, "wol": wo_l,
                "cmask": cmask, "sinT": sinT, "cosT": cosT,
            })
    return in_maps


def _check_causal(mask, seq):
    """True if blocks strictly above the diagonal may be skipped (mask very
    negative -> exp underflows to 0) and blocks at/below the diagonal need
    no mask add (mask exactly 0)."""
    m = mask.reshape(seq, seq)
    iu = np.triu_indices(seq, k=1)
    il = np.tril_indices(seq, k=0)
    return bool((m[iu] <= -1e4).all() and (m[il] == 0.0).all())


def kernel(x, start_pos, inv_freqs, mask, Wq, Wk, Wv, Wo, bo, _trace=False):
    from concourse.bass_utils import run_bass_kernel_spmd

    x = np.asarray(x, dtype=np.float32)
    inv_freqs = np.asarray(inv_freqs, dtype=np.float32)
    mask = np.asarray(mask, dtype=np.float32)
    Wq = np.asarray(Wq, dtype=np.float32)
    Wk = np.asarray(Wk, dtype=np.float32)
    Wv = np.asarray(Wv, dtype=np.float32)
    Wo = np.asarray(Wo, dtype=np.float32)
    bo = np.asarray(bo, dtype=np.float32)

    seq = x.shape[1]
    causal = _check_causal(mask, seq)
    nc = _get_compiled(seq, causal)
    if causal:
        in_maps = _prep_in_maps_causal(x, inv_freqs, Wq, Wk, Wv, Wo, seq)
    else:
        in_maps = _prep_in_maps_generic(x, inv_freqs, mask, Wq, Wk, Wv, Wo,
                                        seq)

    res = run_bass_kernel_spmd(nc, in_maps, core_ids=list(range(8)),
                               trace=_trace)
    outs = [np.asarray(r["out"], dtype=np.float32) for r in res.results]
    y = np.empty((B, seq, C), dtype=np.float32)
    for b in range(B):
        y[b] = outs[2 * b] + outs[2 * b + 1] + bo[None, :]
    if _trace:
        kernel._last_results = res
    return y
